# revision 39
# baseline (speedup 1.0000x reference)
"""Multi-head attention (B=2,S=4096,E=768,H=12,D=64 + 16-token K/V prompt
prefix) on 8 Trainium2 NeuronCores.

Sharding: 2 batches x 4 head-groups (3 heads each). Each core computes QKV
projections for its 3 heads, full attention over its batch, and a partial
output projection (its 192 ctx channels); the host sums the 4 partials per
batch.

v3 design (413us -> 369us): the q/k activations are stored as fp8e4m3
and the scores matmuls run in DoubleRow perf mode (0.5 cycles per output
row -> 2x the bf16 scores throughput). DoubleRow needs a [K, 2, N]
two-k-tile operand shape but the scores contraction is only d=64, so both
j-planes read the SAME data via a stride-0 broadcast dim (psum = 2*kT^T q
exactly; the 0.5 folds into the exp scale). End-to-end rel err ~1.76e-2
(fp8 scores ~1.2e-2 + Schraudolph ~1.2e-2 + bf16 base ~0.45e-2, adding in
quadrature) vs the 2e-2 budget, measured in numpy and on device.

Scores for a kt-PAIR (2x128 kpos) land in one 2-bank psum tile and are
exp'd by a single [128, 1024] activation op, alternating ScalarE (exact
exp) / DVE (Schraudolph bit-trick, F_DVE of tiles) as in v2 but with half
the per-op overhead. ctx stays bf16 (fp8 expt/v would blow the error
budget). All other matmuls bf16. Slot stream is kt-pair-major: 8 sqb x 16
kp x 3 heads = 384 slots.

Per-core layout:
  qT8[d,2,s], kT8[d,2,s] fp8  ([128, pr, S]; DR via stride-0 broadcast)
  v[s,c]           natural + ones col  (denominator in ctx col 64)
  scoresT[k,q]     = DR(kT8-tile, qT8)  (psum [128, 2, 512] = kt-pair)
  expT             = Exp(scores/8)      ([128,1024]: ScalarE | DVE bit-trick)
  ctx[q,c+1]       = expT-tile^T @ v    (psc [128, 4, 256] bank-aligned,
                                         zeroed by first-touch start=True)
  ctx_norm         = ctx * recip(ctx[:,64])
  ctxT             = xbar DMA transpose of ctx_norm head-pairs
  outT[e,q]        = Wo-tiles^T @ ctxT  (partial; host sums 4 groups, bf16)
"""

import sys
import threading

import numpy as np

if "/opt/trn_rl_repo" not in sys.path:
    sys.path.insert(0, "/opt/trn_rl_repo")

import ml_dtypes

BF16 = ml_dtypes.bfloat16
E4M3 = ml_dtypes.float8_e4m3

B, S, E, H, D, PP = 2, 4096, 768, 12, 64, 16
NCORES = 8
NG = 4          # head-groups (tensor parallel)
HL = H // NG    # 3 local heads
CL = HL * D     # 192 local channels
NKT = S // 128  # 32 k-tiles
NKP = NKT // 2  # 16 kt-pairs
SQB = 512       # q block width in the attention stream
NSQB = S // SQB
NST = S // 128  # v s-tiles
QT = 1024       # q width for projection blocks
NQB = S // QT   # 1024-q projection blocks
SLOTS_PER_SQB = HL * NKP        # 48
TRAIL = 15      # ctx trails scores/exp by this many pair-slots
TRAIL0 = 15     # uniform (deep-trail corruption fixed by pop break)
F_DVE = 0.45    # mid-block DVE exp share (see f_dve_at)
F_SQB0 = 0.34   # DVE share in the PE-bound first block
F_LOW = 0.28    # DVE share in the post-boundary congestion window
WLO, WHI = 6, 18  # congestion window within a block
EXTRA_EXPT = 4
OP_DELAY = 10   # slots between norm emission and out-proj matmuls
EPI_DELAY = 2   # slots between bg-proj matmuls and their Act epilogue
POP_HOLD = 4    # MUST exceed the 0..3 norm spread: pops emit the next
                # block's psc matmuls, which must follow the qi3 memset
INIT_HOLD = 10  # first pop waits this many slots (vproj warm-up)
OUT_DVE = 1
VP_PACE = 3
TAPER = 4       # pop pause shrink at the end of the stream
TAPER_AT = 99   # per-block taper disabled (was neutral-to-harmful)
TAPER_POST = 0  # keep the low trail into the next block
TRAIL_MIN = 3
LOG2E = 1.4426950408889634

_lock = threading.Lock()
_compiled = {}


def _build():
    import concourse.bass as bass  # noqa: F401
    import concourse.mybir as mybir
    import concourse.tile as tile
    from concourse import bacc

    f32 = mybir.dt.float32
    bf16 = mybir.dt.bfloat16
    fp8 = mybir.dt.float8e4
    i16 = mybir.dt.int16
    EXP = mybir.ActivationFunctionType.Exp
    MULT = mybir.AluOpType.mult
    ADD = mybir.AluOpType.add
    DR = mybir.MatmulPerfMode.DoubleRow

    nc = bacc.Bacc("TRN2", target_bir_lowering=False, debug=False)

    xqT = nc.dram_tensor("xqT", [E, S], bf16, kind="ExternalInput").ap()
    xkT = nc.dram_tensor("xkT", [E, S], bf16, kind="ExternalInput").ap()
    xvT = nc.dram_tensor("xvT", [E, S], bf16, kind="ExternalInput").ap()
    wqT = nc.dram_tensor("wqT", [E, CL], bf16, kind="ExternalInput").ap()
    wkT = nc.dram_tensor("wkT", [E, CL], bf16, kind="ExternalInput").ap()
    wvT = nc.dram_tensor("wvT", [E, CL], bf16, kind="ExternalInput").ap()
    woT = nc.dram_tensor("woT", [CL, E], bf16, kind="ExternalInput").ap()
    bq = nc.dram_tensor("bq", [CL, 1], f32, kind="ExternalInput").ap()
    bk = nc.dram_tensor("bk", [CL, 1], f32, kind="ExternalInput").ap()
    bv = nc.dram_tensor("bv", [1, CL], f32, kind="ExternalInput").ap()
    kpT = nc.dram_tensor("kpT", [128, 2, PP], fp8, kind="ExternalInput").ap()
    vp = nc.dram_tensor("vp", [128, HL, D + 1], bf16, kind="ExternalInput").ap()
    outT = nc.dram_tensor("outT", [E, S], bf16, kind="ExternalOutput").ap()

    # Schraudolph constants: psum holds raw q.k scores; exp arg = psum/8.
    # bf16 bits = 128*log2(e^(x)) + 16256 => psum * (128*log2e/8) + const.
    SCH_A = 128.0 * LOG2E * 0.0625  # doubled scores
    SCH_B = 16256.0 + 0.5 - 5.8  # +0.5 trunc->round, -5.8 sawtooth centering

    with tile.TileContext(nc) as tc:
        with tc.tile_pool(name="persist", bufs=1) as pers:
            # q-projection weights/bias first: they gate the very first
            # matmuls
            wq_sb = pers.tile([128, 6, CL], bf16)
            nc.gpsimd.dma_start(wq_sb[:], wqT.rearrange("(t p) c -> p t c", p=128))
            bq_sb = pers.tile([128, 2], f32)
            nc.gpsimd.dma_start(bq_sb[:, 0:1], bq[0:128, :])
            nc.gpsimd.dma_start(bq_sb[0:64, 1:2], bq[128:CL, :])

            wk_sb = pers.tile([128, 6, CL], bf16)
            wv_sb = pers.tile([128, 6, CL], bf16)
            wo_sb = pers.tile([128, 2, E], bf16)
            bk_sb = pers.tile([128, 2], f32)
            bvb_sb = pers.tile([128, CL], f32)
            kpT_sb = pers.tile([128, 2, PP], fp8)
            # prefix v: head h at partitions 32h..32h+PP (matches ep rows)
            vp_sb = pers.tile([128, HL, D + 1], bf16)

            # fp8 activations: [partition(d within head-pair), pr, S].
            # DoubleRow reads each operand TWICE via a stride-0 broadcast
            # j-dim, so psum = 2 * kT^T q exactly; the 0.5 folds into the
            # exp scale (0.0625 instead of 0.125).
            qT8_sb = pers.tile([128, 2, S], fp8)
            kT8_sb = pers.tile([128, 2, S], fp8)
            v_sb = pers.tile([128, NST, HL, D + 1], bf16)
            ctxT_sb = pers.tile([128, 2, S], bf16)
            # ctx_norm staging for xbar transposes: [qi, (h0,h1,h2,pad), d]
            ctxn_sb = pers.tile([128, SQB // 128, 4, D], bf16)

            nc.vector.memset(v_sb[:, :, :, D:D + 1], 1.0)
            nc.vector.memset(ctxn_sb[:, :, 3, :], 0.0)

            # ---------------- Phase 1a: Q / K projections ----------------
            with (
                tc.tile_pool(name="ps_proj", bufs=2, space="PSUM") as pp,
                tc.tile_pool(name="xq_pool", bufs=12) as xq_pool,
            ):
                # k/prefix weights on the Pool queue while ALL 12 q+k
                # input chunks stream interleaved on the faster SP/HWDGE
                # queue; non-critical weights (wv/wo/prefix-v) follow on
                # Pool after
                nc.gpsimd.dma_start(
                    wk_sb[:], wkT.rearrange("(t p) c -> p t c", p=128))
                nc.gpsimd.dma_start(bk_sb[:, 0:1], bk[0:128, :])
                nc.gpsimd.dma_start(bk_sb[0:64, 1:2], bk[128:CL, :])
                nc.gpsimd.dma_start(kpT_sb[:], kpT[:])

                # ALL 12 q+k input chunks stream interleaved on the
                # faster SP/HWDGE queue
                xts = {}
                for ech in range(6):
                    for which, xin in (("q", xqT), ("k", xkT)):
                        xt = xq_pool.tile([128, QT], bf16, tag="xt",
                                          name="xt")
                        nc.sync.dma_start(
                            xt[:], xin[ech * 128:(ech + 1) * 128, 0:QT])
                        xts[(which, ech)] = xt

                def proj_block0(which, wsb, bsb, dst):
                    p0 = pp.tile([128, QT], f32, tag="p0", name="p0")
                    p1 = pp.tile([64, QT], f32, tag="p1", name="p1")
                    for ech in range(6):
                        xt = xts[(which, ech)]
                        for n in range(QT // 512):
                            ns = slice(n * 512, (n + 1) * 512)
                            nc.tensor.matmul(
                                p0[:, ns], wsb[:, ech, 0:128], xt[:, ns],
                                start=(ech == 0), stop=(ech == 5),
                            )
                            nc.tensor.matmul(
                                p1[:, ns], wsb[:, ech, 128:CL], xt[:, ns],
                                start=(ech == 0), stop=(ech == 5),
                            )
                    for n in range(QT // 512):
                        ds = slice(n * 512, (n + 1) * 512)
                        ns = slice(n * 512, (n + 1) * 512)
                        nc.scalar.add(dst[:, 0, ds], p0[:, ns], bsb[:, 0:1])
                        nc.scalar.add(dst[0:64, 1, ds], p1[0:64, ns],
                                      bsb[0:64, 1:2])

                proj_block0("q", wq_sb, bq_sb, qT8_sb)
                proj_block0("k", wk_sb, bk_sb, kT8_sb)
                nc.gpsimd.dma_start(
                    wv_sb[:], wvT.rearrange("(t p) c -> p t c", p=128))
                nc.gpsimd.dma_start(bvb_sb[:], bv.to_broadcast((128, CL)))
                nc.gpsimd.dma_start(vp_sb[:], vp[:])
                nc.gpsimd.dma_start(wo_sb[:, 0, :], woT[0:128, :])
                nc.gpsimd.dma_start(wo_sb[0:64, 1, :], woT[128:CL, :])

            # ---------- attention stream ----------
            # slot order is kp-major: for each sq-block, sweep kt-pairs in
            # order with the 3 heads innermost. Background k/q projections
            # and the v-projection are spread across the stream so the DMA
            # engines never burst.
            # One unified 3-deep psum ring (tag "u", 2-bank slots = 6 banks)
            # serves scores pairs AND the proj/vproj/prefix/outproj scratch;
            # psc takes the other 2 banks. Depth 3 hides the
            # matmul->exp->psum-free turnaround that a 2-deep ring cannot.
            with (
                tc.tile_pool(name="ps_str", bufs=3, space="PSUM") as ps_str,
                tc.tile_pool(name="ps_acc", bufs=1, space="PSUM") as ps_acc,
                tc.tile_pool(name="expt_pool",
                             bufs=max(TRAIL, TRAIL0) + POP_HOLD
                             + EXTRA_EXPT) as expt_pool,
                tc.tile_pool(name="expp_pool", bufs=2) as expp_pool,
                tc.tile_pool(name="rc_pool", bufs=4) as rc_pool,
                tc.tile_pool(name="xv_pool", bufs=8) as xv_pool,
                tc.tile_pool(name="xq2_pool", bufs=7) as xq2_pool,
                tc.tile_pool(name="xk2_pool", bufs=12) as xk2_pool,
                tc.tile_pool(name="out_pool", bufs=4) as out_pool,
            ):
                # ctx accumulator: 2 banks; slice (qi, h) = [128q, 65].
                # 12 interleaved accumulation chains share the banks, so
                # matmul start=True (bank-granular zeroing) cannot be used:
                # the tile is DVE-memset per sq-block and every ctx matmul
                # accumulates with start=False.
                # [128, qi, 256]: each qi stride 1KB, 2 qi per psum bank.
                # Explicitly memset between blocks (on DVE, in-order after
                # the norm reads) -- matmul start=True bank-zeroing would
                # race with the deferred norm reads of the previous block.
                psc = ps_acc.tile([128, SQB // 128, 256], f32, name="psc")
                nc.vector.memset(psc[:], 0.0)

                # ---- background projections, chunk-granular ----
                # each (c, grp) group is split into a matmul op and a
                # deferred Act epilogue so the Act in-order queue never
                # parks on a bias-add whose matmuls are still running
                def make_bg_proj(xin, wsb, bsb, dst, sq, dma_eng, pool):
                    mm_ops = []
                    epi_ops = []
                    state = {}

                    def mk_dma(ech):
                        def op():
                            xt2 = pool.tile([128, QT], bf16, tag="xt2",
                                            name="xt2")
                            dma_eng.dma_start(
                                xt2[:],
                                xin[ech * 128:(ech + 1) * 128,
                                    sq * QT:(sq + 1) * QT],
                            )
                            state[ech] = xt2
                        return op

                    def mk_group(c, grp):
                        def mm_op():
                            pt = ps_str.tile([128, 512], f32, tag="u",
                                             name="pq")
                            rows = 128 if grp == 0 else 64
                            wc = slice(0, 128) if grp == 0 else slice(128, CL)
                            for ech in range(6):
                                nc.tensor.matmul(
                                    pt[0:rows, :], wsb[:, ech, wc],
                                    state[ech][:, c * 512:(c + 1) * 512],
                                    start=(ech == 0), stop=(ech == 5),
                                )
                            state[("pt", c, grp)] = pt

                        def epi_op():
                            pt = state.pop(("pt", c, grp))
                            qs = slice(sq * QT + c * 512,
                                       sq * QT + (c + 1) * 512)
                            if grp == 0:
                                nc.scalar.add(
                                    dst[:, 0, qs], pt[:, :], bsb[:, 0:1])
                            else:
                                nc.scalar.add(
                                    dst[0:64, 1, qs], pt[0:64, :],
                                    bsb[0:64, 1:2])
                        return mm_op, epi_op

                    for ech in range(6):
                        mm_ops.append(mk_dma(ech))
                    for c in range(QT // 512):
                        for grp in range(2):
                            mm, epi = mk_group(c, grp)
                            mm_ops.append(mm)
                            epi_ops.append(epi)
                    return mm_ops, epi_ops

                bg_work = []
                # k blocks 1..: block b first needed at pair-slot 12b
                for sq in range(1, NQB):
                    base = 12 * (sq - 1)
                    mm_ops, epi_ops = make_bg_proj(
                        xkT, wk_sb, bk_sb, kT8_sb, sq, nc.sync, xk2_pool)
                    tags = [base + i for i in range(6)] + \
                           [base + 7, base + 8, base + 9, base + 10]
                    for tg, op in zip(tags, mm_ops):
                        bg_work.append((tg, op))
                    for i, op in enumerate(epi_ops):
                        bg_work.append((base + 7 + i + EPI_DELAY, op))
                # q blocks 1..: block n needed by pair-slot 96n
                for sq in range(1, NQB):
                    t0 = max(14, 2 * sq * SLOTS_PER_SQB - 70)
                    mm_ops, epi_ops = make_bg_proj(
                        xqT, wq_sb, bq_sb, qT8_sb, sq, nc.gpsimd, xq2_pool)
                    for i, op in enumerate(mm_ops):
                        bg_work.append((t0 + i, op))
                    for i, op in enumerate(epi_ops):
                        bg_work.append((t0 + 6 + i + EPI_DELAY, op))
                bg_work.sort(key=lambda x: x[0])

                # ---- V projection (natural orientation) ----
                xvts = {}

                def load_xv_chunk(sqx, ech, eng=None):
                    xvt = xv_pool.tile([128, QT], bf16, tag="xvt",
                                       name="xvt")
                    (eng or nc.gpsimd).dma_start(
                        xvt[:],
                        xvT[ech * 128:(ech + 1) * 128,
                            sqx * QT:(sqx + 1) * QT],
                    )
                    xvts[(sqx, ech)] = xvt

                def emit_vproj_pair(stp):
                    # two s-tiles (2*stp, 2*stp+1) share one psum tile
                    pv = ps_str.tile([128, 2, 512], f32, tag="u", name="pv")
                    for half in range(2):
                        st = 2 * stp + half
                        sqx, stl = st // (QT // 128), st % (QT // 128)
                        if st == 0:
                            for ech in range(6):
                                load_xv_chunk(0, ech, nc.sync)
                        if stl < 6 and sqx + 1 < NQB:
                            load_xv_chunk(sqx + 1, stl)
                        for ech in range(6):
                            nc.tensor.matmul(
                                pv[:, half, 0:CL],
                                xvts[(sqx, ech)][:,
                                                 stl * 128:(stl + 1) * 128],
                                wv_sb[:, ech, :],
                                start=(ech == 0), stop=(ech == 5),
                            )
                    for half in range(2):
                        st = 2 * stp + half
                        sqx, stl = st // (QT // 128), st % (QT // 128)
                        nc.vector.tensor_add(
                            v_sb[:, st, :, 0:D],
                            pv[:, half, 0:CL].rearrange(
                                "p (h d) -> p h d", h=HL),
                            bvb_sb[:].rearrange("p (h d) -> p h d", h=HL),
                        )
                        if stl == (QT // 128) - 1:
                            for ech in range(6):
                                del xvts[(sqx, ech)]

                # ---- prefix scores + exp for one sq-block (3 heads) ----
                expp_cur = [None]

                def emit_prefix(sqb):
                    psm = ps_str.tile([128, 512], f32, tag="u", name="psp")
                    qs = slice(sqb * SQB, (sqb + 1) * SQB)
                    # head h lives at partitions 32h..32h+16 (same rows as
                    # its scores psum), so one [80, 512] activation handles
                    # all three heads; rows 16-31/48-63 are junk but finite
                    ep = expp_pool.tile([128, SQB], bf16, tag="ep",
                                        name="ep")
                    for h in range(HL):
                        pr, po = h // 2, 64 * (h % 2)
                        nc.tensor.matmul(
                            psm[32 * h:32 * h + PP, :],
                            kpT_sb[po:po + 64, pr, :],
                            qT8_sb[po:po + 64, pr, qs],
                            start=True, stop=True,
                        )
                    if sqb == 0:
                        # ring psum is uninitialized at t=0: only touch the
                        # rows the matmuls wrote
                        for h in range(HL):
                            nc.scalar.activation(
                                ep[32 * h:32 * h + PP, :],
                                psm[32 * h:32 * h + PP, :],
                                EXP, scale=0.125)
                    else:
                        nc.scalar.activation(
                            ep[0:80, :], psm[0:80, :], EXP, scale=0.125)
                    expp_cur[0] = ep

                # ---- scores (DoubleRow fp8) + exp for one (sqb, kp, h) ----
                # dynamic Schraudolph share: Act absorbs more exp where DVE
                # is congested (right after a block's norm) and in the
                # PE-bound first block; DVE takes more mid-block
                dve_acc = [0.0]

                def f_dve_at(t):
                    if t < SLOTS_PER_SQB + 6:
                        return F_SQB0
                    p = t % SLOTS_PER_SQB
                    if WLO <= p < WHI:
                        return F_LOW
                    return F_DVE

                def emit_scores_exp(t, sqb, kp, h):
                    pr, po = h // 2, 64 * (h % 2)
                    qs = slice(sqb * SQB, (sqb + 1) * SQB)
                    pss = ps_str.tile([128, 2, 512], f32, tag="u",
                                      name="pss")
                    qmov = qT8_sb[po:po + 64, pr:pr + 1, qs].to_broadcast(
                        (64, 2, SQB))
                    for i in range(2):
                        kt = 2 * kp + i
                        nc.tensor.matmul(
                            pss[:, i, :],
                            kT8_sb[po:po + 64, pr:pr + 1,
                                   kt * 128:(kt + 1) * 128].to_broadcast(
                                (64, 2, 128)),
                            qmov,
                            start=True, stop=True,
                            perf_mode=DR,
                        )
                    expt = expt_pool.tile([128, 2, 512], bf16, tag="expt",
                                          name="expt")
                    # near-alternating engine assignment so consecutive exps
                    # overlap across ScalarE / DVE
                    use_dve = False
                    dve_acc[0] += f_dve_at(t)
                    if dve_acc[0] >= 1.0:
                        dve_acc[0] -= 1.0
                        use_dve = True
                    if use_dve:
                        nc.vector.tensor_scalar(
                            expt[:].bitcast(i16), pss[:], SCH_A, SCH_B,
                            MULT, ADD)
                    else:
                        nc.scalar.activation(expt[:], pss[:], EXP,
                                             scale=0.0625)
                    return expt

                # ---- ctx (natural orientation) ----
                def emit_ctx(sqb, kp, h, expt, ep):
                    for qi in range(SQB // 128):
                        acc = psc[:, qi, 65 * h:65 * h + 65]
                        if kp == 0:
                            nc.tensor.matmul(
                                acc,
                                ep[32 * h:32 * h + PP,
                                   qi * 128:(qi + 1) * 128],
                                vp_sb[32 * h:32 * h + PP, h, :],
                                start=False, stop=False,
                                skip_group_check=True,
                            )
                        for i in range(2):
                            kt = 2 * kp + i
                            nc.tensor.matmul(
                                acc,
                                expt[:, i, qi * 128:(qi + 1) * 128],
                                v_sb[:, kt, h, :],
                                start=False, stop=False,
                                skip_group_check=True,
                            )

                # ---- norm + transpose + out-projection per sq-block ----
                outproj_work = []

                norm_work = []

                def emit_norm_qi(sqb, qi):
                    pscq = psc[:, qi, 0:195].rearrange(
                        "p (h c) -> p h c", c=65)
                    rc = rc_pool.tile([128, 4], f32, tag="rc", name="rc")
                    nc.vector.reciprocal(
                        rc[:, 0:HL], pscq[:, :, D])
                    for h in range(HL):
                        nc.vector.tensor_scalar_mul(
                            ctxn_sb[:, qi, h, :],
                            pscq[:, h, 0:D],
                            rc[:, h:h + 1])
                    qs = slice(sqb * SQB + qi * 128,
                               sqb * SQB + (qi + 1) * 128)
                    nc.sync.dma_start(
                        ctxT_sb[:, 0, qs], ctxn_sb[:, qi, 0:2, :],
                        transpose=True)
                    nc.sync.dma_start(
                        ctxT_sb[:, 1, qs], ctxn_sb[:, qi, 2:4, :],
                        transpose=True)
                    if qi == SQB // 128 - 1:
                        nc.vector.memset(psc[:], 0.0)
                        # delay the out-projection matmuls so their
                        # transpose dependencies clear before they enter
                        # PE's in-order queue
                        for et in range(6):
                            outproj_work.append(
                                (cur_t[0] + OP_DELAY + 2 * et, et, sqb))

                def emit_norm(sqb):
                    for qi in range(SQB // 128):
                        norm_work.append((cur_t[0] + qi, sqb, qi))

                def emit_outproj_tile(due, et, sqb):
                    es = slice(et * 128, (et + 1) * 128)
                    qs = slice(sqb * SQB, (sqb + 1) * SQB)
                    po3 = ps_str.tile([128, 512], f32, tag="u", name="po3")
                    for qi in range(SQB // 128):
                        ns = slice(qi * 128, (qi + 1) * 128)
                        qs2 = slice(sqb * SQB + qi * 128,
                                    sqb * SQB + (qi + 1) * 128)
                        nc.tensor.matmul(
                            po3[:, ns], wo_sb[:, 0, es], ctxT_sb[:, 0, qs2],
                            start=True, stop=False,
                        )
                        nc.tensor.matmul(
                            po3[:, ns], wo_sb[0:64, 1, es],
                            ctxT_sb[0:64, 1, qs2],
                            start=False, stop=True,
                        )
                    ot = out_pool.tile([128, 512], bf16, tag="ot", name="ot")
                    if OUT_DVE and sqb < NSQB - 1 and et % 2 == 0:
                        nc.vector.tensor_copy(ot[:], po3[:])
                    else:
                        nc.scalar.copy(ot[:], po3[:])
                    nc.gpsimd.dma_start(outT[es, qs], ot[:])

                # ---- the slot stream ----
                slots = [(sqb, kp, h)
                         for sqb in range(NSQB)
                         for kp in range(NKP)
                         for h in range(HL)]
                pending = []
                vst = 0
                cur_t = [0]
                hold_until = [INIT_HOLD]

                def pop_one():
                    (s2, e2, ep2) = pending.pop(0)
                    sqb2, kp2, h2 = s2
                    emit_ctx(sqb2, kp2, h2, e2, ep2)
                    if kp2 == NKP - 1 and h2 == HL - 1:
                        emit_norm(sqb2)
                        hold_until[0] = cur_t[0] + POP_HOLD
                        return True
                    return False

                for t, slot in enumerate(slots):
                    cur_t[0] = t
                    sqb, kp, h = slot
                    if kp == 0 and h == 0:
                        emit_prefix(sqb)
                    expt = emit_scores_exp(t, *slot)
                    pending.append((slot, expt, expp_cur[0]))
                    if vst < NST // 2 and t % VP_PACE == 0:
                        emit_vproj_pair(vst)
                        vst += 1
                    if t < 60:
                        trail_eff = TRAIL0
                    elif t < len(slots) - TAPER:
                        # per-block taper: drain the trail during each
                        # block's last slots (PE-light there) so the norm
                        # lands at the boundary, not 8 slots into the next
                        p = t % SLOTS_PER_SQB
                        if p >= TAPER_AT or p < TAPER_POST:
                            trail_eff = TRAIL_MIN
                        else:
                            trail_eff = TRAIL
                    else:
                        trail_eff = 1
                    if t >= hold_until[0]:
                        for _ in range(3):
                            if len(pending) > trail_eff:
                                # stop popping the moment a block ends: the
                                # next block's psc matmuls must not be
                                # emitted before the deferred norm + memset
                                if pop_one():
                                    break
                            else:
                                break
                    while norm_work and norm_work[0][0] <= t:
                        emit_norm_qi(*norm_work.pop(0)[1:])
                    while bg_work and bg_work[0][0] <= t:
                        bg_work.pop(0)[1]()
                    if outproj_work and outproj_work[0][0] <= t:
                        emit_outproj_tile(*outproj_work.pop(0))
                while pending:
                    pop_one()
                    while norm_work:
                        emit_norm_qi(*norm_work.pop(0)[1:])
                    if outproj_work:
                        emit_outproj_tile(*outproj_work.pop(0))
                while norm_work:
                    emit_norm_qi(*norm_work.pop(0)[1:])
                for _, op in bg_work:
                    op()
                while outproj_work:
                    emit_outproj_tile(*outproj_work.pop(0))

    nc.compile()
    return nc


def _get_nc():
    with _lock:
        if "nc" not in _compiled:
            _compiled["nc"] = _build()
        return _compiled["nc"]


def _prep_in_maps(query, key, value, prompt, Wq, bq, Wk, bk, Wv, bv, Wo, bo):
    f32 = np.float32
    qT = [np.ascontiguousarray(query[b].T).astype(BF16) for b in range(B)]
    kT = [np.ascontiguousarray(key[b].T).astype(BF16) for b in range(B)]
    vT = [np.ascontiguousarray(value[b].T).astype(BF16) for b in range(B)]
    in_maps = []
    for core in range(NCORES):
        b, g = core // NG, core % NG
        cs = slice(g * CL, (g + 1) * CL)
        kp = np.zeros((128, 2, PP), E4M3)
        vpa = np.zeros((128, HL, D + 1), BF16)
        for h in range(HL):
            gh = g * HL + h
            kp[64 * (h % 2):64 * (h % 2) + 64, h // 2, :] = (
                prompt[b, 0, :, gh, :].T.astype(E4M3))
            vpa[32 * h:32 * h + PP, h, D] = 1.0
            vpa[32 * h:32 * h + PP, h, 0:D] = (
                prompt[b, 1, :, gh, :].astype(BF16))
        in_maps.append({
            "xqT": qT[b], "xkT": kT[b], "xvT": vT[b],
            "wqT": np.ascontiguousarray(Wq[cs, :].T).astype(BF16),
            "wkT": np.ascontiguousarray(Wk[cs, :].T).astype(BF16),
            "wvT": np.ascontiguousarray(Wv[cs, :].T).astype(BF16),
            "woT": np.ascontiguousarray(Wo[:, cs].T).astype(BF16),
            "bq": np.ascontiguousarray(bq[cs]).astype(f32).reshape(CL, 1),
            "bk": np.ascontiguousarray(bk[cs]).astype(f32).reshape(CL, 1),
            "bv": np.ascontiguousarray(bv[cs]).astype(f32).reshape(1, CL),
            "kpT": kp, "vp": vpa,
        })
    return in_maps


def _combine(results, bo):
    out = np.empty((B, S, E), np.float32)
    for b in range(B):
        acc = results[b * NG]["outT"].astype(np.float32)
        for g in range(1, NG):
            acc = acc + results[b * NG + g]["outT"].astype(np.float32)
        out[b] = acc.T
    if bo is not None and np.any(bo):
        out += np.asarray(bo, np.float32)
    return out


def run(inputs, trace=False):
    """Returns (output, exec_time_ns or None)."""
    from concourse import bass_utils

    nc = _get_nc()
    in_maps = _prep_in_maps(**{k: np.asarray(v) for k, v in inputs.items()})
    bo = np.asarray(inputs["bo"])
    res = bass_utils.run_bass_kernel_spmd(
        nc, in_maps, core_ids=list(range(NCORES)), trace=trace,
    )
    return _combine(res.results, bo), res.exec_time_ns


def kernel(**inputs):
    out, _ = run(inputs)
    return out


# revision 41
# speedup vs baseline: 1.0059x; 1.0059x over previous
"""Multi-head attention (B=2,S=4096,E=768,H=12,D=64 + 16-token K/V prompt
prefix) on 8 Trainium2 NeuronCores.

Sharding: 2 batches x 4 head-groups (3 heads each). Each core computes QKV
projections for its 3 heads, full attention over its batch, and a partial
output projection (its 192 ctx channels); the host sums the 4 partials per
batch.

v3 design (413us -> 369us): the q/k activations are stored as fp8e4m3
and the scores matmuls run in DoubleRow perf mode (0.5 cycles per output
row -> 2x the bf16 scores throughput). DoubleRow needs a [K, 2, N]
two-k-tile operand shape but the scores contraction is only d=64, so both
j-planes read the SAME data via a stride-0 broadcast dim (psum = 2*kT^T q
exactly; the 0.5 folds into the exp scale). End-to-end rel err ~1.76e-2
(fp8 scores ~1.2e-2 + Schraudolph ~1.2e-2 + bf16 base ~0.45e-2, adding in
quadrature) vs the 2e-2 budget, measured in numpy and on device.

Scores for a kt-PAIR (2x128 kpos) land in one 2-bank psum tile and are
exp'd by a single [128, 1024] activation op, alternating ScalarE (exact
exp) / DVE (Schraudolph bit-trick, F_DVE of tiles) as in v2 but with half
the per-op overhead. ctx stays bf16 (fp8 expt/v would blow the error
budget). All other matmuls bf16. Slot stream is kt-pair-major: 8 sqb x 16
kp x 3 heads = 384 slots.

Per-core layout:
  qT8[d,2,s], kT8[d,2,s] fp8  ([128, pr, S]; DR via stride-0 broadcast)
  v[s,c]           natural + ones col  (denominator in ctx col 64)
  scoresT[k,q]     = DR(kT8-tile, qT8)  (psum [128, 2, 512] = kt-pair)
  expT             = Exp(scores/8)      ([128,1024]: ScalarE | DVE bit-trick)
  ctx[q,c+1]       = expT-tile^T @ v    (psc [128, 4, 256] bank-aligned,
                                         zeroed by first-touch start=True)
  ctx_norm         = ctx * recip(ctx[:,64])
  ctxT             = xbar DMA transpose of ctx_norm head-pairs
  outT[e,q]        = Wo-tiles^T @ ctxT  (partial; host sums 4 groups, bf16)
"""

import sys
import threading

import numpy as np

if "/opt/trn_rl_repo" not in sys.path:
    sys.path.insert(0, "/opt/trn_rl_repo")

import ml_dtypes

BF16 = ml_dtypes.bfloat16
E4M3 = ml_dtypes.float8_e4m3

B, S, E, H, D, PP = 2, 4096, 768, 12, 64, 16
NCORES = 8
NG = 4          # head-groups (tensor parallel)
HL = H // NG    # 3 local heads
CL = HL * D     # 192 local channels
NKT = S // 128  # 32 k-tiles
NKP = NKT // 2  # 16 kt-pairs
SQB = 512       # q block width in the attention stream
NSQB = S // SQB
NST = S // 128  # v s-tiles
QT = 1024       # q width for projection blocks
NQB = S // QT   # 1024-q projection blocks
SLOTS_PER_SQB = HL * NKP        # 48
TRAIL = 15      # ctx trails scores/exp by this many pair-slots
TRAIL0 = 15     # uniform (deep-trail corruption fixed by pop break)
F_DVE = 0.45    # mid-block DVE exp share (see f_dve_at)
F_SQB0 = 0.34   # DVE share in the PE-bound first block
F_LOW = 0.28    # DVE share in the post-boundary congestion window
WLO, WHI = 6, 18  # congestion window within a block
EXTRA_EXPT = 4
OP_DELAY = 10   # slots between norm emission and out-proj matmuls
EPI_DELAY = 2   # slots between bg-proj matmuls and their Act epilogue
POP_HOLD = 4    # MUST exceed the 0..3 norm spread: pops emit the next
                # block's psc matmuls, which must follow the qi3 memset
INIT_HOLD = 10  # first pop waits this many slots (vproj warm-up)
OUT_DVE = 1
VP_PACE = 3
TAPER = 4       # pop pause shrink at the end of the stream
TAPER_AT = 99   # per-block taper disabled (was neutral-to-harmful)
TAPER_POST = 0  # keep the low trail into the next block
QBG_OFF = -90   # bg q-proj placement relative to its deadline
TRAIL_MIN = 3
LOG2E = 1.4426950408889634

_lock = threading.Lock()
_compiled = {}


def _build():
    import concourse.bass as bass  # noqa: F401
    import concourse.mybir as mybir
    import concourse.tile as tile
    from concourse import bacc

    f32 = mybir.dt.float32
    bf16 = mybir.dt.bfloat16
    fp8 = mybir.dt.float8e4
    i16 = mybir.dt.int16
    EXP = mybir.ActivationFunctionType.Exp
    MULT = mybir.AluOpType.mult
    ADD = mybir.AluOpType.add
    DR = mybir.MatmulPerfMode.DoubleRow

    nc = bacc.Bacc("TRN2", target_bir_lowering=False, debug=False)

    xqT = nc.dram_tensor("xqT", [E, S], bf16, kind="ExternalInput").ap()
    xkT = nc.dram_tensor("xkT", [E, S], bf16, kind="ExternalInput").ap()
    xvT = nc.dram_tensor("xvT", [E, S], bf16, kind="ExternalInput").ap()
    wqT = nc.dram_tensor("wqT", [E, CL], bf16, kind="ExternalInput").ap()
    wkT = nc.dram_tensor("wkT", [E, CL], bf16, kind="ExternalInput").ap()
    wvT = nc.dram_tensor("wvT", [E, CL], bf16, kind="ExternalInput").ap()
    woT = nc.dram_tensor("woT", [CL, E], bf16, kind="ExternalInput").ap()
    bq = nc.dram_tensor("bq", [CL, 1], f32, kind="ExternalInput").ap()
    bk = nc.dram_tensor("bk", [CL, 1], f32, kind="ExternalInput").ap()
    bv = nc.dram_tensor("bv", [1, CL], f32, kind="ExternalInput").ap()
    kpT = nc.dram_tensor("kpT", [128, 2, PP], fp8, kind="ExternalInput").ap()
    vp = nc.dram_tensor("vp", [128, HL, D + 1], bf16, kind="ExternalInput").ap()
    outT = nc.dram_tensor("outT", [E, S], bf16, kind="ExternalOutput").ap()

    # Schraudolph constants: psum holds raw q.k scores; exp arg = psum/8.
    # bf16 bits = 128*log2(e^(x)) + 16256 => psum * (128*log2e/8) + const.
    SCH_A = 128.0 * LOG2E * 0.0625  # doubled scores
    SCH_B = 16256.0 + 0.5 - 5.8  # +0.5 trunc->round, -5.8 sawtooth centering

    with tile.TileContext(nc) as tc:
        with tc.tile_pool(name="persist", bufs=1) as pers:
            # q-projection weights/bias first: they gate the very first
            # matmuls
            wq_sb = pers.tile([128, 6, CL], bf16)
            nc.gpsimd.dma_start(wq_sb[:], wqT.rearrange("(t p) c -> p t c", p=128))
            bq_sb = pers.tile([128, 2], f32)
            nc.gpsimd.dma_start(bq_sb[:, 0:1], bq[0:128, :])
            nc.gpsimd.dma_start(bq_sb[0:64, 1:2], bq[128:CL, :])

            wk_sb = pers.tile([128, 6, CL], bf16)
            wv_sb = pers.tile([128, 6, CL], bf16)
            wo_sb = pers.tile([128, 2, E], bf16)
            bk_sb = pers.tile([128, 2], f32)
            bvb_sb = pers.tile([128, CL], f32)
            kpT_sb = pers.tile([128, 2, PP], fp8)
            # prefix v: head h at partitions 32h..32h+PP (matches ep rows)
            vp_sb = pers.tile([128, HL, D + 1], bf16)

            # fp8 activations: [partition(d within head-pair), pr, S].
            # DoubleRow reads each operand TWICE via a stride-0 broadcast
            # j-dim, so psum = 2 * kT^T q exactly; the 0.5 folds into the
            # exp scale (0.0625 instead of 0.125).
            qT8_sb = pers.tile([128, 2, S], fp8)
            kT8_sb = pers.tile([128, 2, S], fp8)
            v_sb = pers.tile([128, NST, HL, D + 1], bf16)
            ctxT_sb = pers.tile([128, 2, S], bf16)
            # ctx_norm staging for xbar transposes: [qi, (h0,h1,h2,pad), d]
            ctxn_sb = pers.tile([128, SQB // 128, 4, D], bf16)

            nc.vector.memset(v_sb[:, :, :, D:D + 1], 1.0)
            nc.vector.memset(ctxn_sb[:, :, 3, :], 0.0)

            # ---------------- Phase 1a: Q / K projections ----------------
            with (
                tc.tile_pool(name="ps_proj", bufs=2, space="PSUM") as pp,
                tc.tile_pool(name="xq_pool", bufs=12) as xq_pool,
            ):
                # k/prefix weights on the Pool queue while ALL 12 q+k
                # input chunks stream interleaved on the faster SP/HWDGE
                # queue; non-critical weights (wv/wo/prefix-v) follow on
                # Pool after
                nc.gpsimd.dma_start(
                    wk_sb[:], wkT.rearrange("(t p) c -> p t c", p=128))
                nc.gpsimd.dma_start(bk_sb[:, 0:1], bk[0:128, :])
                nc.gpsimd.dma_start(bk_sb[0:64, 1:2], bk[128:CL, :])
                nc.gpsimd.dma_start(kpT_sb[:], kpT[:])

                # ALL 12 q+k input chunks stream interleaved on the
                # faster SP/HWDGE queue
                xts = {}
                for ech in range(6):
                    for which, xin in (("q", xqT), ("k", xkT)):
                        xt = xq_pool.tile([128, QT], bf16, tag="xt",
                                          name="xt")
                        nc.sync.dma_start(
                            xt[:], xin[ech * 128:(ech + 1) * 128, 0:QT])
                        xts[(which, ech)] = xt

                def proj_block0(which, wsb, bsb, dst):
                    p0 = pp.tile([128, QT], f32, tag="p0", name="p0")
                    p1 = pp.tile([64, QT], f32, tag="p1", name="p1")
                    for ech in range(6):
                        xt = xts[(which, ech)]
                        for n in range(QT // 512):
                            ns = slice(n * 512, (n + 1) * 512)
                            nc.tensor.matmul(
                                p0[:, ns], wsb[:, ech, 0:128], xt[:, ns],
                                start=(ech == 0), stop=(ech == 5),
                            )
                            nc.tensor.matmul(
                                p1[:, ns], wsb[:, ech, 128:CL], xt[:, ns],
                                start=(ech == 0), stop=(ech == 5),
                            )
                    for n in range(QT // 512):
                        ds = slice(n * 512, (n + 1) * 512)
                        ns = slice(n * 512, (n + 1) * 512)
                        nc.scalar.add(dst[:, 0, ds], p0[:, ns], bsb[:, 0:1])
                        nc.scalar.add(dst[0:64, 1, ds], p1[0:64, ns],
                                      bsb[0:64, 1:2])

                proj_block0("q", wq_sb, bq_sb, qT8_sb)
                proj_block0("k", wk_sb, bk_sb, kT8_sb)
                nc.gpsimd.dma_start(
                    wv_sb[:], wvT.rearrange("(t p) c -> p t c", p=128))
                nc.gpsimd.dma_start(bvb_sb[:], bv.to_broadcast((128, CL)))
                nc.gpsimd.dma_start(vp_sb[:], vp[:])
                nc.gpsimd.dma_start(wo_sb[:, 0, :], woT[0:128, :])
                nc.gpsimd.dma_start(wo_sb[0:64, 1, :], woT[128:CL, :])

            # ---------- attention stream ----------
            # slot order is kp-major: for each sq-block, sweep kt-pairs in
            # order with the 3 heads innermost. Background k/q projections
            # and the v-projection are spread across the stream so the DMA
            # engines never burst.
            # One unified 3-deep psum ring (tag "u", 2-bank slots = 6 banks)
            # serves scores pairs AND the proj/vproj/prefix/outproj scratch;
            # psc takes the other 2 banks. Depth 3 hides the
            # matmul->exp->psum-free turnaround that a 2-deep ring cannot.
            with (
                tc.tile_pool(name="ps_str", bufs=3, space="PSUM") as ps_str,
                tc.tile_pool(name="ps_acc", bufs=1, space="PSUM") as ps_acc,
                tc.tile_pool(name="expt_pool",
                             bufs=max(TRAIL, TRAIL0) + POP_HOLD
                             + EXTRA_EXPT) as expt_pool,
                tc.tile_pool(name="expp_pool", bufs=2) as expp_pool,
                tc.tile_pool(name="rc_pool", bufs=4) as rc_pool,
                tc.tile_pool(name="xv_pool", bufs=8) as xv_pool,
                tc.tile_pool(name="xq2_pool", bufs=7) as xq2_pool,
                tc.tile_pool(name="xk2_pool", bufs=12) as xk2_pool,
                tc.tile_pool(name="out_pool", bufs=4) as out_pool,
            ):
                # ctx accumulator: 2 banks; slice (qi, h) = [128q, 65].
                # 12 interleaved accumulation chains share the banks, so
                # matmul start=True (bank-granular zeroing) cannot be used:
                # the tile is DVE-memset per sq-block and every ctx matmul
                # accumulates with start=False.
                # [128, qi, 256]: each qi stride 1KB, 2 qi per psum bank.
                # Explicitly memset between blocks (on DVE, in-order after
                # the norm reads) -- matmul start=True bank-zeroing would
                # race with the deferred norm reads of the previous block.
                psc = ps_acc.tile([128, SQB // 128, 256], f32, name="psc")
                nc.vector.memset(psc[:], 0.0)

                # ---- background projections, chunk-granular ----
                # each (c, grp) group is split into a matmul op and a
                # deferred Act epilogue so the Act in-order queue never
                # parks on a bias-add whose matmuls are still running
                def make_bg_proj(xin, wsb, bsb, dst, sq, dma_eng, pool):
                    mm_ops = []
                    epi_ops = []
                    state = {}

                    def mk_dma(ech):
                        def op():
                            xt2 = pool.tile([128, QT], bf16, tag="xt2",
                                            name="xt2")
                            dma_eng.dma_start(
                                xt2[:],
                                xin[ech * 128:(ech + 1) * 128,
                                    sq * QT:(sq + 1) * QT],
                            )
                            state[ech] = xt2
                        return op

                    def mk_group(c, grp):
                        def mm_op():
                            pt = ps_str.tile([128, 512], f32, tag="u",
                                             name="pq")
                            rows = 128 if grp == 0 else 64
                            wc = slice(0, 128) if grp == 0 else slice(128, CL)
                            for ech in range(6):
                                nc.tensor.matmul(
                                    pt[0:rows, :], wsb[:, ech, wc],
                                    state[ech][:, c * 512:(c + 1) * 512],
                                    start=(ech == 0), stop=(ech == 5),
                                )
                            state[("pt", c, grp)] = pt

                        def epi_op():
                            pt = state.pop(("pt", c, grp))
                            qs = slice(sq * QT + c * 512,
                                       sq * QT + (c + 1) * 512)
                            if grp == 0:
                                nc.scalar.add(
                                    dst[:, 0, qs], pt[:, :], bsb[:, 0:1])
                            else:
                                nc.scalar.add(
                                    dst[0:64, 1, qs], pt[0:64, :],
                                    bsb[0:64, 1:2])
                        return mm_op, epi_op

                    for ech in range(6):
                        mm_ops.append(mk_dma(ech))
                    for c in range(QT // 512):
                        for grp in range(2):
                            mm, epi = mk_group(c, grp)
                            mm_ops.append(mm)
                            epi_ops.append(epi)
                    return mm_ops, epi_ops

                bg_work = []
                # k blocks 1..: block b first needed at pair-slot 12b
                for sq in range(1, NQB):
                    base = 12 * (sq - 1)
                    mm_ops, epi_ops = make_bg_proj(
                        xkT, wk_sb, bk_sb, kT8_sb, sq, nc.sync, xk2_pool)
                    tags = [base + i for i in range(6)] + \
                           [base + 7, base + 8, base + 9, base + 10]
                    for tg, op in zip(tags, mm_ops):
                        bg_work.append((tg, op))
                    for i, op in enumerate(epi_ops):
                        bg_work.append((base + 7 + i + EPI_DELAY, op))
                # q blocks 1..: block n needed by pair-slot 96n
                for sq in range(1, NQB):
                    t0 = max(14, 2 * sq * SLOTS_PER_SQB + QBG_OFF)
                    mm_ops, epi_ops = make_bg_proj(
                        xqT, wq_sb, bq_sb, qT8_sb, sq, nc.gpsimd, xq2_pool)
                    for i, op in enumerate(mm_ops):
                        bg_work.append((t0 + i, op))
                    for i, op in enumerate(epi_ops):
                        bg_work.append((t0 + 6 + i + EPI_DELAY, op))
                bg_work.sort(key=lambda x: x[0])

                # ---- V projection (natural orientation) ----
                xvts = {}

                def load_xv_chunk(sqx, ech, eng=None):
                    xvt = xv_pool.tile([128, QT], bf16, tag="xvt",
                                       name="xvt")
                    (eng or nc.gpsimd).dma_start(
                        xvt[:],
                        xvT[ech * 128:(ech + 1) * 128,
                            sqx * QT:(sqx + 1) * QT],
                    )
                    xvts[(sqx, ech)] = xvt

                def emit_vproj_pair(stp):
                    # two s-tiles (2*stp, 2*stp+1) share one psum tile
                    pv = ps_str.tile([128, 2, 512], f32, tag="u", name="pv")
                    for half in range(2):
                        st = 2 * stp + half
                        sqx, stl = st // (QT // 128), st % (QT // 128)
                        if st == 0:
                            for ech in range(6):
                                load_xv_chunk(0, ech, nc.sync)
                        if stl < 6 and sqx + 1 < NQB:
                            load_xv_chunk(sqx + 1, stl)
                        for ech in range(6):
                            nc.tensor.matmul(
                                pv[:, half, 0:CL],
                                xvts[(sqx, ech)][:,
                                                 stl * 128:(stl + 1) * 128],
                                wv_sb[:, ech, :],
                                start=(ech == 0), stop=(ech == 5),
                            )
                    for half in range(2):
                        st = 2 * stp + half
                        sqx, stl = st // (QT // 128), st % (QT // 128)
                        nc.vector.tensor_add(
                            v_sb[:, st, :, 0:D],
                            pv[:, half, 0:CL].rearrange(
                                "p (h d) -> p h d", h=HL),
                            bvb_sb[:].rearrange("p (h d) -> p h d", h=HL),
                        )
                        if stl == (QT // 128) - 1:
                            for ech in range(6):
                                del xvts[(sqx, ech)]

                # ---- prefix scores + exp for one sq-block (3 heads) ----
                expp_cur = [None]

                def emit_prefix(sqb):
                    psm = ps_str.tile([128, 512], f32, tag="u", name="psp")
                    qs = slice(sqb * SQB, (sqb + 1) * SQB)
                    # head h lives at partitions 32h..32h+16 (same rows as
                    # its scores psum), so one [80, 512] activation handles
                    # all three heads; rows 16-31/48-63 are junk but finite
                    ep = expp_pool.tile([128, SQB], bf16, tag="ep",
                                        name="ep")
                    for h in range(HL):
                        pr, po = h // 2, 64 * (h % 2)
                        nc.tensor.matmul(
                            psm[32 * h:32 * h + PP, :],
                            kpT_sb[po:po + 64, pr, :],
                            qT8_sb[po:po + 64, pr, qs],
                            start=True, stop=True,
                        )
                    if sqb == 0:
                        # ring psum is uninitialized at t=0: only touch the
                        # rows the matmuls wrote
                        for h in range(HL):
                            nc.scalar.activation(
                                ep[32 * h:32 * h + PP, :],
                                psm[32 * h:32 * h + PP, :],
                                EXP, scale=0.125)
                    else:
                        nc.scalar.activation(
                            ep[0:80, :], psm[0:80, :], EXP, scale=0.125)
                    expp_cur[0] = ep

                # ---- scores (DoubleRow fp8) + exp for one (sqb, kp, h) ----
                # dynamic Schraudolph share: Act absorbs more exp where DVE
                # is congested (right after a block's norm) and in the
                # PE-bound first block; DVE takes more mid-block
                dve_acc = [0.0]

                def f_dve_at(t):
                    if t < SLOTS_PER_SQB + 6:
                        return F_SQB0
                    p = t % SLOTS_PER_SQB
                    if WLO <= p < WHI:
                        return F_LOW
                    return F_DVE

                def emit_scores_exp(t, sqb, kp, h):
                    pr, po = h // 2, 64 * (h % 2)
                    qs = slice(sqb * SQB, (sqb + 1) * SQB)
                    pss = ps_str.tile([128, 2, 512], f32, tag="u",
                                      name="pss")
                    qmov = qT8_sb[po:po + 64, pr:pr + 1, qs].to_broadcast(
                        (64, 2, SQB))
                    for i in range(2):
                        kt = 2 * kp + i
                        nc.tensor.matmul(
                            pss[:, i, :],
                            kT8_sb[po:po + 64, pr:pr + 1,
                                   kt * 128:(kt + 1) * 128].to_broadcast(
                                (64, 2, 128)),
                            qmov,
                            start=True, stop=True,
                            perf_mode=DR,
                        )
                    expt = expt_pool.tile([128, 2, 512], bf16, tag="expt",
                                          name="expt")
                    # near-alternating engine assignment so consecutive exps
                    # overlap across ScalarE / DVE
                    use_dve = False
                    dve_acc[0] += f_dve_at(t)
                    if dve_acc[0] >= 1.0:
                        dve_acc[0] -= 1.0
                        use_dve = True
                    if use_dve:
                        nc.vector.tensor_scalar(
                            expt[:].bitcast(i16), pss[:], SCH_A, SCH_B,
                            MULT, ADD)
                    else:
                        nc.scalar.activation(expt[:], pss[:], EXP,
                                             scale=0.0625)
                    return expt

                # ---- ctx (natural orientation) ----
                def emit_ctx(sqb, kp, h, expt, ep):
                    for qi in range(SQB // 128):
                        acc = psc[:, qi, 65 * h:65 * h + 65]
                        if kp == 0:
                            nc.tensor.matmul(
                                acc,
                                ep[32 * h:32 * h + PP,
                                   qi * 128:(qi + 1) * 128],
                                vp_sb[32 * h:32 * h + PP, h, :],
                                start=False, stop=False,
                                skip_group_check=True,
                            )
                        for i in range(2):
                            kt = 2 * kp + i
                            nc.tensor.matmul(
                                acc,
                                expt[:, i, qi * 128:(qi + 1) * 128],
                                v_sb[:, kt, h, :],
                                start=False, stop=False,
                                skip_group_check=True,
                            )

                # ---- norm + transpose + out-projection per sq-block ----
                outproj_work = []

                norm_work = []

                def emit_norm_qi(sqb, qi):
                    pscq = psc[:, qi, 0:195].rearrange(
                        "p (h c) -> p h c", c=65)
                    rc = rc_pool.tile([128, 4], f32, tag="rc", name="rc")
                    nc.vector.reciprocal(
                        rc[:, 0:HL], pscq[:, :, D])
                    for h in range(HL):
                        nc.vector.tensor_scalar_mul(
                            ctxn_sb[:, qi, h, :],
                            pscq[:, h, 0:D],
                            rc[:, h:h + 1])
                    qs = slice(sqb * SQB + qi * 128,
                               sqb * SQB + (qi + 1) * 128)
                    nc.sync.dma_start(
                        ctxT_sb[:, 0, qs], ctxn_sb[:, qi, 0:2, :],
                        transpose=True)
                    nc.sync.dma_start(
                        ctxT_sb[:, 1, qs], ctxn_sb[:, qi, 2:4, :],
                        transpose=True)
                    if qi == SQB // 128 - 1:
                        nc.vector.memset(psc[:], 0.0)
                        # delay the out-projection matmuls so their
                        # transpose dependencies clear before they enter
                        # PE's in-order queue
                        for et in range(6):
                            outproj_work.append(
                                (cur_t[0] + OP_DELAY + 2 * et, et, sqb))

                def emit_norm(sqb):
                    for qi in range(SQB // 128):
                        norm_work.append((cur_t[0] + qi, sqb, qi))

                def emit_outproj_tile(due, et, sqb):
                    es = slice(et * 128, (et + 1) * 128)
                    qs = slice(sqb * SQB, (sqb + 1) * SQB)
                    po3 = ps_str.tile([128, 512], f32, tag="u", name="po3")
                    for qi in range(SQB // 128):
                        ns = slice(qi * 128, (qi + 1) * 128)
                        qs2 = slice(sqb * SQB + qi * 128,
                                    sqb * SQB + (qi + 1) * 128)
                        nc.tensor.matmul(
                            po3[:, ns], wo_sb[:, 0, es], ctxT_sb[:, 0, qs2],
                            start=True, stop=False,
                        )
                        nc.tensor.matmul(
                            po3[:, ns], wo_sb[0:64, 1, es],
                            ctxT_sb[0:64, 1, qs2],
                            start=False, stop=True,
                        )
                    ot = out_pool.tile([128, 512], bf16, tag="ot", name="ot")
                    if OUT_DVE and sqb < NSQB - 1 and et % 2 == 0:
                        nc.vector.tensor_copy(ot[:], po3[:])
                    else:
                        nc.scalar.copy(ot[:], po3[:])
                    nc.gpsimd.dma_start(outT[es, qs], ot[:])

                # ---- the slot stream ----
                slots = [(sqb, kp, h)
                         for sqb in range(NSQB)
                         for kp in range(NKP)
                         for h in range(HL)]
                pending = []
                vst = 0
                cur_t = [0]
                hold_until = [INIT_HOLD]

                def pop_one():
                    (s2, e2, ep2) = pending.pop(0)
                    sqb2, kp2, h2 = s2
                    emit_ctx(sqb2, kp2, h2, e2, ep2)
                    if kp2 == NKP - 1 and h2 == HL - 1:
                        emit_norm(sqb2)
                        hold_until[0] = cur_t[0] + POP_HOLD
                        return True
                    return False

                for t, slot in enumerate(slots):
                    cur_t[0] = t
                    sqb, kp, h = slot
                    if kp == 0 and h == 0:
                        emit_prefix(sqb)
                    expt = emit_scores_exp(t, *slot)
                    pending.append((slot, expt, expp_cur[0]))
                    if vst < NST // 2 and t % VP_PACE == 0:
                        emit_vproj_pair(vst)
                        vst += 1
                    if t < 60:
                        trail_eff = TRAIL0
                    elif t < len(slots) - TAPER:
                        # per-block taper: drain the trail during each
                        # block's last slots (PE-light there) so the norm
                        # lands at the boundary, not 8 slots into the next
                        p = t % SLOTS_PER_SQB
                        if p >= TAPER_AT or p < TAPER_POST:
                            trail_eff = TRAIL_MIN
                        else:
                            trail_eff = TRAIL
                    else:
                        trail_eff = 1
                    if t >= hold_until[0]:
                        for _ in range(3):
                            if len(pending) > trail_eff:
                                # stop popping the moment a block ends: the
                                # next block's psc matmuls must not be
                                # emitted before the deferred norm + memset
                                if pop_one():
                                    break
                            else:
                                break
                    while norm_work and norm_work[0][0] <= t:
                        emit_norm_qi(*norm_work.pop(0)[1:])
                    while bg_work and bg_work[0][0] <= t:
                        bg_work.pop(0)[1]()
                    if outproj_work and outproj_work[0][0] <= t:
                        emit_outproj_tile(*outproj_work.pop(0))
                while pending:
                    pop_one()
                    while norm_work:
                        emit_norm_qi(*norm_work.pop(0)[1:])
                    if outproj_work:
                        emit_outproj_tile(*outproj_work.pop(0))
                while norm_work:
                    emit_norm_qi(*norm_work.pop(0)[1:])
                for _, op in bg_work:
                    op()
                while outproj_work:
                    emit_outproj_tile(*outproj_work.pop(0))

    nc.compile()
    return nc


def _get_nc():
    with _lock:
        if "nc" not in _compiled:
            _compiled["nc"] = _build()
        return _compiled["nc"]


def _prep_in_maps(query, key, value, prompt, Wq, bq, Wk, bk, Wv, bv, Wo, bo):
    f32 = np.float32
    qT = [np.ascontiguousarray(query[b].T).astype(BF16) for b in range(B)]
    kT = [np.ascontiguousarray(key[b].T).astype(BF16) for b in range(B)]
    vT = [np.ascontiguousarray(value[b].T).astype(BF16) for b in range(B)]
    in_maps = []
    for core in range(NCORES):
        b, g = core // NG, core % NG
        cs = slice(g * CL, (g + 1) * CL)
        kp = np.zeros((128, 2, PP), E4M3)
        vpa = np.zeros((128, HL, D + 1), BF16)
        for h in range(HL):
            gh = g * HL + h
            kp[64 * (h % 2):64 * (h % 2) + 64, h // 2, :] = (
                prompt[b, 0, :, gh, :].T.astype(E4M3))
            vpa[32 * h:32 * h + PP, h, D] = 1.0
            vpa[32 * h:32 * h + PP, h, 0:D] = (
                prompt[b, 1, :, gh, :].astype(BF16))
        in_maps.append({
            "xqT": qT[b], "xkT": kT[b], "xvT": vT[b],
            "wqT": np.ascontiguousarray(Wq[cs, :].T).astype(BF16),
            "wkT": np.ascontiguousarray(Wk[cs, :].T).astype(BF16),
            "wvT": np.ascontiguousarray(Wv[cs, :].T).astype(BF16),
            "woT": np.ascontiguousarray(Wo[:, cs].T).astype(BF16),
            "bq": np.ascontiguousarray(bq[cs]).astype(f32).reshape(CL, 1),
            "bk": np.ascontiguousarray(bk[cs]).astype(f32).reshape(CL, 1),
            "bv": np.ascontiguousarray(bv[cs]).astype(f32).reshape(1, CL),
            "kpT": kp, "vp": vpa,
        })
    return in_maps


def _combine(results, bo):
    out = np.empty((B, S, E), np.float32)
    for b in range(B):
        acc = results[b * NG]["outT"].astype(np.float32)
        for g in range(1, NG):
            acc = acc + results[b * NG + g]["outT"].astype(np.float32)
        out[b] = acc.T
    if bo is not None and np.any(bo):
        out += np.asarray(bo, np.float32)
    return out


def run(inputs, trace=False):
    """Returns (output, exec_time_ns or None)."""
    from concourse import bass_utils

    nc = _get_nc()
    in_maps = _prep_in_maps(**{k: np.asarray(v) for k, v in inputs.items()})
    bo = np.asarray(inputs["bo"])
    res = bass_utils.run_bass_kernel_spmd(
        nc, in_maps, core_ids=list(range(NCORES)), trace=trace,
    )
    return _combine(res.results, bo), res.exec_time_ns


def kernel(**inputs):
    out, _ = run(inputs)
    return out


# revision 44
# speedup vs baseline: 1.0082x; 1.0022x over previous
"""Multi-head attention (B=2,S=4096,E=768,H=12,D=64 + 16-token K/V prompt
prefix) on 8 Trainium2 NeuronCores.

Sharding: 2 batches x 4 head-groups (3 heads each). Each core computes QKV
projections for its 3 heads, full attention over its batch, and a partial
output projection (its 192 ctx channels); the host sums the 4 partials per
batch.

v3 design (413us -> 369us): the q/k activations are stored as fp8e4m3
and the scores matmuls run in DoubleRow perf mode (0.5 cycles per output
row -> 2x the bf16 scores throughput). DoubleRow needs a [K, 2, N]
two-k-tile operand shape but the scores contraction is only d=64, so both
j-planes read the SAME data via a stride-0 broadcast dim (psum = 2*kT^T q
exactly; the 0.5 folds into the exp scale). End-to-end rel err ~1.76e-2
(fp8 scores ~1.2e-2 + Schraudolph ~1.2e-2 + bf16 base ~0.45e-2, adding in
quadrature) vs the 2e-2 budget, measured in numpy and on device.

Scores for a kt-PAIR (2x128 kpos) land in one 2-bank psum tile and are
exp'd by a single [128, 1024] activation op, alternating ScalarE (exact
exp) / DVE (Schraudolph bit-trick, F_DVE of tiles) as in v2 but with half
the per-op overhead. ctx stays bf16 (fp8 expt/v would blow the error
budget). All other matmuls bf16. Slot stream is kt-pair-major: 8 sqb x 16
kp x 3 heads = 384 slots.

Per-core layout:
  qT8[d,2,s], kT8[d,2,s] fp8  ([128, pr, S]; DR via stride-0 broadcast)
  v[s,c]           natural + ones col  (denominator in ctx col 64)
  scoresT[k,q]     = DR(kT8-tile, qT8)  (psum [128, 2, 512] = kt-pair)
  expT             = Exp(scores/8)      ([128,1024]: ScalarE | DVE bit-trick)
  ctx[q,c+1]       = expT-tile^T @ v    (psc [128, 4, 256] bank-aligned,
                                         zeroed by first-touch start=True)
  ctx_norm         = ctx * recip(ctx[:,64])
  ctxT             = xbar DMA transpose of ctx_norm head-pairs
  outT[e,q]        = Wo-tiles^T @ ctxT  (partial; host sums 4 groups, bf16)
"""

import sys
import threading

import numpy as np

if "/opt/trn_rl_repo" not in sys.path:
    sys.path.insert(0, "/opt/trn_rl_repo")

import ml_dtypes

BF16 = ml_dtypes.bfloat16
E4M3 = ml_dtypes.float8_e4m3

B, S, E, H, D, PP = 2, 4096, 768, 12, 64, 16
NCORES = 8
NG = 4          # head-groups (tensor parallel)
HL = H // NG    # 3 local heads
CL = HL * D     # 192 local channels
NKT = S // 128  # 32 k-tiles
NKP = NKT // 2  # 16 kt-pairs
SQB = 512       # q block width in the attention stream
NSQB = S // SQB
NST = S // 128  # v s-tiles
QT = 1024       # q width for projection blocks
NQB = S // QT   # 1024-q projection blocks
SLOTS_PER_SQB = HL * NKP        # 48
TRAIL = 15      # ctx trails scores/exp by this many pair-slots
TRAIL0 = 15     # uniform (deep-trail corruption fixed by pop break)
F_DVE = 0.45    # mid-block DVE exp share (see f_dve_at)
F_SQB0 = 0.34   # DVE share in the PE-bound first block
F_LOW = 0.28    # DVE share in the post-boundary congestion window
WLO, WHI = 6, 18  # congestion window within a block
EXTRA_EXPT = 4
OP_DELAY = 10   # slots between norm emission and out-proj matmuls
EPI_DELAY = 2   # slots between bg-proj matmuls and their Act epilogue
POP_HOLD = 4    # MUST exceed the 0..3 norm spread: pops emit the next
                # block's psc matmuls, which must follow the qi3 memset
INIT_HOLD = 10  # first pop waits this many slots (vproj warm-up)
OUT_DVE = 1
VP_PACE = 3
TAPER = 4       # pop pause shrink at the end of the stream
TAPER_AT = 99   # per-block taper disabled (was neutral-to-harmful)
TAPER_POST = 0  # keep the low trail into the next block
QBG_OFF = -90   # bg q-proj placement relative to its deadline
OP_SP = 2       # out-proj tile spacing
KG_OFF = 9      # k-bg group position within its window
TRAIL_MIN = 3
LOG2E = 1.4426950408889634

_lock = threading.Lock()
_compiled = {}


def _build():
    import concourse.bass as bass  # noqa: F401
    import concourse.mybir as mybir
    import concourse.tile as tile
    from concourse import bacc

    f32 = mybir.dt.float32
    bf16 = mybir.dt.bfloat16
    fp8 = mybir.dt.float8e4
    i16 = mybir.dt.int16
    EXP = mybir.ActivationFunctionType.Exp
    MULT = mybir.AluOpType.mult
    ADD = mybir.AluOpType.add
    DR = mybir.MatmulPerfMode.DoubleRow

    nc = bacc.Bacc("TRN2", target_bir_lowering=False, debug=False)

    xqT = nc.dram_tensor("xqT", [E, S], bf16, kind="ExternalInput").ap()
    xkT = nc.dram_tensor("xkT", [E, S], bf16, kind="ExternalInput").ap()
    xvT = nc.dram_tensor("xvT", [E, S], bf16, kind="ExternalInput").ap()
    wqT = nc.dram_tensor("wqT", [E, CL], bf16, kind="ExternalInput").ap()
    wkT = nc.dram_tensor("wkT", [E, CL], bf16, kind="ExternalInput").ap()
    wvT = nc.dram_tensor("wvT", [E, CL], bf16, kind="ExternalInput").ap()
    woT = nc.dram_tensor("woT", [CL, E], bf16, kind="ExternalInput").ap()
    bq = nc.dram_tensor("bq", [CL, 1], f32, kind="ExternalInput").ap()
    bk = nc.dram_tensor("bk", [CL, 1], f32, kind="ExternalInput").ap()
    bv = nc.dram_tensor("bv", [1, CL], f32, kind="ExternalInput").ap()
    kpT = nc.dram_tensor("kpT", [128, 2, PP], fp8, kind="ExternalInput").ap()
    vp = nc.dram_tensor("vp", [128, HL, D + 1], bf16, kind="ExternalInput").ap()
    outT = nc.dram_tensor("outT", [E, S], bf16, kind="ExternalOutput").ap()

    # Schraudolph constants: psum holds raw q.k scores; exp arg = psum/8.
    # bf16 bits = 128*log2(e^(x)) + 16256 => psum * (128*log2e/8) + const.
    SCH_A = 128.0 * LOG2E * 0.0625  # doubled scores
    SCH_B = 16256.0 + 0.5 - 5.8  # +0.5 trunc->round, -5.8 sawtooth centering

    with tile.TileContext(nc) as tc:
        with tc.tile_pool(name="persist", bufs=1) as pers:
            # q-projection weights/bias first: they gate the very first
            # matmuls
            wq_sb = pers.tile([128, 6, CL], bf16)
            nc.gpsimd.dma_start(wq_sb[:], wqT.rearrange("(t p) c -> p t c", p=128))
            bq_sb = pers.tile([128, 2], f32)
            nc.gpsimd.dma_start(bq_sb[:, 0:1], bq[0:128, :])
            nc.gpsimd.dma_start(bq_sb[0:64, 1:2], bq[128:CL, :])

            wk_sb = pers.tile([128, 6, CL], bf16)
            wv_sb = pers.tile([128, 6, CL], bf16)
            wo_sb = pers.tile([128, 2, E], bf16)
            bk_sb = pers.tile([128, 2], f32)
            bvb_sb = pers.tile([128, CL], f32)
            kpT_sb = pers.tile([128, 2, PP], fp8)
            # prefix v: head h at partitions 32h..32h+PP (matches ep rows)
            vp_sb = pers.tile([128, HL, D + 1], bf16)

            # fp8 activations: [partition(d within head-pair), pr, S].
            # DoubleRow reads each operand TWICE via a stride-0 broadcast
            # j-dim, so psum = 2 * kT^T q exactly; the 0.5 folds into the
            # exp scale (0.0625 instead of 0.125).
            qT8_sb = pers.tile([128, 2, S], fp8)
            kT8_sb = pers.tile([128, 2, S], fp8)
            v_sb = pers.tile([128, NST, HL, D + 1], bf16)
            ctxT_sb = pers.tile([128, 2, S], bf16)
            # ctx_norm staging for xbar transposes: [qi, (h0,h1,h2,pad), d]
            ctxn_sb = pers.tile([128, SQB // 128, 4, D], bf16)

            nc.vector.memset(v_sb[:, :, :, D:D + 1], 1.0)
            nc.vector.memset(ctxn_sb[:, :, 3, :], 0.0)

            # ---------------- Phase 1a: Q / K projections ----------------
            with (
                tc.tile_pool(name="ps_proj", bufs=2, space="PSUM") as pp,
                tc.tile_pool(name="xq_pool", bufs=12) as xq_pool,
            ):
                # k/prefix weights on the Pool queue while ALL 12 q+k
                # input chunks stream interleaved on the faster SP/HWDGE
                # queue; non-critical weights (wv/wo/prefix-v) follow on
                # Pool after
                nc.gpsimd.dma_start(
                    wk_sb[:], wkT.rearrange("(t p) c -> p t c", p=128))
                nc.gpsimd.dma_start(bk_sb[:, 0:1], bk[0:128, :])
                nc.gpsimd.dma_start(bk_sb[0:64, 1:2], bk[128:CL, :])
                nc.gpsimd.dma_start(kpT_sb[:], kpT[:])

                # ALL 12 q+k input chunks stream interleaved on the
                # faster SP/HWDGE queue
                xts = {}
                for ech in range(6):
                    for which, xin in (("q", xqT), ("k", xkT)):
                        xt = xq_pool.tile([128, QT], bf16, tag="xt",
                                          name="xt")
                        nc.sync.dma_start(
                            xt[:], xin[ech * 128:(ech + 1) * 128, 0:QT])
                        xts[(which, ech)] = xt

                def proj_block0(which, wsb, bsb, dst):
                    p0 = pp.tile([128, QT], f32, tag="p0", name="p0")
                    p1 = pp.tile([64, QT], f32, tag="p1", name="p1")
                    for ech in range(6):
                        xt = xts[(which, ech)]
                        for n in range(QT // 512):
                            ns = slice(n * 512, (n + 1) * 512)
                            nc.tensor.matmul(
                                p0[:, ns], wsb[:, ech, 0:128], xt[:, ns],
                                start=(ech == 0), stop=(ech == 5),
                            )
                            nc.tensor.matmul(
                                p1[:, ns], wsb[:, ech, 128:CL], xt[:, ns],
                                start=(ech == 0), stop=(ech == 5),
                            )
                    for n in range(QT // 512):
                        ds = slice(n * 512, (n + 1) * 512)
                        ns = slice(n * 512, (n + 1) * 512)
                        nc.scalar.add(dst[:, 0, ds], p0[:, ns], bsb[:, 0:1])
                        nc.scalar.add(dst[0:64, 1, ds], p1[0:64, ns],
                                      bsb[0:64, 1:2])

                proj_block0("q", wq_sb, bq_sb, qT8_sb)
                proj_block0("k", wk_sb, bk_sb, kT8_sb)
                nc.gpsimd.dma_start(
                    wv_sb[:], wvT.rearrange("(t p) c -> p t c", p=128))
                nc.gpsimd.dma_start(bvb_sb[:], bv.to_broadcast((128, CL)))
                nc.gpsimd.dma_start(vp_sb[:], vp[:])
                nc.gpsimd.dma_start(wo_sb[:, 0, :], woT[0:128, :])
                nc.gpsimd.dma_start(wo_sb[0:64, 1, :], woT[128:CL, :])

            # ---------- attention stream ----------
            # slot order is kp-major: for each sq-block, sweep kt-pairs in
            # order with the 3 heads innermost. Background k/q projections
            # and the v-projection are spread across the stream so the DMA
            # engines never burst.
            # One unified 3-deep psum ring (tag "u", 2-bank slots = 6 banks)
            # serves scores pairs AND the proj/vproj/prefix/outproj scratch;
            # psc takes the other 2 banks. Depth 3 hides the
            # matmul->exp->psum-free turnaround that a 2-deep ring cannot.
            with (
                tc.tile_pool(name="ps_str", bufs=3, space="PSUM") as ps_str,
                tc.tile_pool(name="ps_acc", bufs=1, space="PSUM") as ps_acc,
                tc.tile_pool(name="expt_pool",
                             bufs=max(TRAIL, TRAIL0) + POP_HOLD
                             + EXTRA_EXPT) as expt_pool,
                tc.tile_pool(name="expp_pool", bufs=2) as expp_pool,
                tc.tile_pool(name="rc_pool", bufs=4) as rc_pool,
                tc.tile_pool(name="xv_pool", bufs=8) as xv_pool,
                tc.tile_pool(name="xq2_pool", bufs=7) as xq2_pool,
                tc.tile_pool(name="xk2_pool", bufs=12) as xk2_pool,
                tc.tile_pool(name="out_pool", bufs=4) as out_pool,
            ):
                # ctx accumulator: 2 banks; slice (qi, h) = [128q, 65].
                # 12 interleaved accumulation chains share the banks, so
                # matmul start=True (bank-granular zeroing) cannot be used:
                # the tile is DVE-memset per sq-block and every ctx matmul
                # accumulates with start=False.
                # [128, qi, 256]: each qi stride 1KB, 2 qi per psum bank.
                # Explicitly memset between blocks (on DVE, in-order after
                # the norm reads) -- matmul start=True bank-zeroing would
                # race with the deferred norm reads of the previous block.
                psc = ps_acc.tile([128, SQB // 128, 256], f32, name="psc")
                nc.vector.memset(psc[:], 0.0)

                # ---- background projections, chunk-granular ----
                # each (c, grp) group is split into a matmul op and a
                # deferred Act epilogue so the Act in-order queue never
                # parks on a bias-add whose matmuls are still running
                def make_bg_proj(xin, wsb, bsb, dst, sq, dma_eng, pool):
                    mm_ops = []
                    epi_ops = []
                    state = {}

                    def mk_dma(ech):
                        def op():
                            xt2 = pool.tile([128, QT], bf16, tag="xt2",
                                            name="xt2")
                            dma_eng.dma_start(
                                xt2[:],
                                xin[ech * 128:(ech + 1) * 128,
                                    sq * QT:(sq + 1) * QT],
                            )
                            state[ech] = xt2
                        return op

                    def mk_group(c, grp):
                        def mm_op():
                            pt = ps_str.tile([128, 512], f32, tag="u",
                                             name="pq")
                            rows = 128 if grp == 0 else 64
                            wc = slice(0, 128) if grp == 0 else slice(128, CL)
                            for ech in range(6):
                                nc.tensor.matmul(
                                    pt[0:rows, :], wsb[:, ech, wc],
                                    state[ech][:, c * 512:(c + 1) * 512],
                                    start=(ech == 0), stop=(ech == 5),
                                )
                            state[("pt", c, grp)] = pt

                        def epi_op():
                            pt = state.pop(("pt", c, grp))
                            qs = slice(sq * QT + c * 512,
                                       sq * QT + (c + 1) * 512)
                            if grp == 0:
                                nc.scalar.add(
                                    dst[:, 0, qs], pt[:, :], bsb[:, 0:1])
                            else:
                                nc.scalar.add(
                                    dst[0:64, 1, qs], pt[0:64, :],
                                    bsb[0:64, 1:2])
                        return mm_op, epi_op

                    for ech in range(6):
                        mm_ops.append(mk_dma(ech))
                    for c in range(QT // 512):
                        for grp in range(2):
                            mm, epi = mk_group(c, grp)
                            mm_ops.append(mm)
                            epi_ops.append(epi)
                    return mm_ops, epi_ops

                bg_work = []
                # k blocks 1..: block b first needed at pair-slot 12b
                for sq in range(1, NQB):
                    base = 12 * (sq - 1)
                    mm_ops, epi_ops = make_bg_proj(
                        xkT, wk_sb, bk_sb, kT8_sb, sq, nc.sync, xk2_pool)
                    tags = [base + i for i in range(6)] + \
                           [base + KG_OFF + i for i in range(4)]
                    for tg, op in zip(tags, mm_ops):
                        bg_work.append((tg, op))
                    for i, op in enumerate(epi_ops):
                        bg_work.append((base + KG_OFF + i + EPI_DELAY, op))
                # q blocks 1..: block n needed by pair-slot 96n
                for sq in range(1, NQB):
                    t0 = max(14, 2 * sq * SLOTS_PER_SQB + QBG_OFF)
                    mm_ops, epi_ops = make_bg_proj(
                        xqT, wq_sb, bq_sb, qT8_sb, sq, nc.gpsimd, xq2_pool)
                    for i, op in enumerate(mm_ops):
                        bg_work.append((t0 + i, op))
                    for i, op in enumerate(epi_ops):
                        bg_work.append((t0 + 6 + i + EPI_DELAY, op))
                bg_work.sort(key=lambda x: x[0])

                # ---- V projection (natural orientation) ----
                xvts = {}

                def load_xv_chunk(sqx, ech, eng=None):
                    xvt = xv_pool.tile([128, QT], bf16, tag="xvt",
                                       name="xvt")
                    (eng or nc.gpsimd).dma_start(
                        xvt[:],
                        xvT[ech * 128:(ech + 1) * 128,
                            sqx * QT:(sqx + 1) * QT],
                    )
                    xvts[(sqx, ech)] = xvt

                def emit_vproj_pair(stp):
                    # two s-tiles (2*stp, 2*stp+1) share one psum tile
                    pv = ps_str.tile([128, 2, 512], f32, tag="u", name="pv")
                    for half in range(2):
                        st = 2 * stp + half
                        sqx, stl = st // (QT // 128), st % (QT // 128)
                        if st == 0:
                            for ech in range(6):
                                load_xv_chunk(0, ech, nc.sync)
                        if stl < 6 and sqx + 1 < NQB:
                            load_xv_chunk(sqx + 1, stl)
                        for ech in range(6):
                            nc.tensor.matmul(
                                pv[:, half, 0:CL],
                                xvts[(sqx, ech)][:,
                                                 stl * 128:(stl + 1) * 128],
                                wv_sb[:, ech, :],
                                start=(ech == 0), stop=(ech == 5),
                            )
                    for half in range(2):
                        st = 2 * stp + half
                        sqx, stl = st // (QT // 128), st % (QT // 128)
                        nc.vector.tensor_add(
                            v_sb[:, st, :, 0:D],
                            pv[:, half, 0:CL].rearrange(
                                "p (h d) -> p h d", h=HL),
                            bvb_sb[:].rearrange("p (h d) -> p h d", h=HL),
                        )
                        if stl == (QT // 128) - 1:
                            for ech in range(6):
                                del xvts[(sqx, ech)]

                # ---- prefix scores + exp for one sq-block (3 heads) ----
                expp_cur = [None]

                def emit_prefix(sqb):
                    psm = ps_str.tile([128, 512], f32, tag="u", name="psp")
                    qs = slice(sqb * SQB, (sqb + 1) * SQB)
                    # head h lives at partitions 32h..32h+16 (same rows as
                    # its scores psum), so one [80, 512] activation handles
                    # all three heads; rows 16-31/48-63 are junk but finite
                    ep = expp_pool.tile([128, SQB], bf16, tag="ep",
                                        name="ep")
                    for h in range(HL):
                        pr, po = h // 2, 64 * (h % 2)
                        nc.tensor.matmul(
                            psm[32 * h:32 * h + PP, :],
                            kpT_sb[po:po + 64, pr, :],
                            qT8_sb[po:po + 64, pr, qs],
                            start=True, stop=True,
                        )
                    if sqb == 0:
                        # ring psum is uninitialized at t=0: only touch the
                        # rows the matmuls wrote
                        for h in range(HL):
                            nc.scalar.activation(
                                ep[32 * h:32 * h + PP, :],
                                psm[32 * h:32 * h + PP, :],
                                EXP, scale=0.125)
                    else:
                        nc.scalar.activation(
                            ep[0:80, :], psm[0:80, :], EXP, scale=0.125)
                    expp_cur[0] = ep

                # ---- scores (DoubleRow fp8) + exp for one (sqb, kp, h) ----
                # dynamic Schraudolph share: Act absorbs more exp where DVE
                # is congested (right after a block's norm) and in the
                # PE-bound first block; DVE takes more mid-block
                dve_acc = [0.0]

                def f_dve_at(t):
                    if t < SLOTS_PER_SQB + 6:
                        return F_SQB0
                    p = t % SLOTS_PER_SQB
                    if WLO <= p < WHI:
                        return F_LOW
                    return F_DVE

                def emit_scores_exp(t, sqb, kp, h):
                    pr, po = h // 2, 64 * (h % 2)
                    qs = slice(sqb * SQB, (sqb + 1) * SQB)
                    pss = ps_str.tile([128, 2, 512], f32, tag="u",
                                      name="pss")
                    qmov = qT8_sb[po:po + 64, pr:pr + 1, qs].to_broadcast(
                        (64, 2, SQB))
                    for i in range(2):
                        kt = 2 * kp + i
                        nc.tensor.matmul(
                            pss[:, i, :],
                            kT8_sb[po:po + 64, pr:pr + 1,
                                   kt * 128:(kt + 1) * 128].to_broadcast(
                                (64, 2, 128)),
                            qmov,
                            start=True, stop=True,
                            perf_mode=DR,
                        )
                    expt = expt_pool.tile([128, 2, 512], bf16, tag="expt",
                                          name="expt")
                    # near-alternating engine assignment so consecutive exps
                    # overlap across ScalarE / DVE
                    use_dve = False
                    dve_acc[0] += f_dve_at(t)
                    if dve_acc[0] >= 1.0:
                        dve_acc[0] -= 1.0
                        use_dve = True
                    if use_dve:
                        nc.vector.tensor_scalar(
                            expt[:].bitcast(i16), pss[:], SCH_A, SCH_B,
                            MULT, ADD)
                    else:
                        nc.scalar.activation(expt[:], pss[:], EXP,
                                             scale=0.0625)
                    return expt

                # ---- ctx (natural orientation) ----
                def emit_ctx(sqb, kp, h, expt, ep):
                    for qi in range(SQB // 128):
                        acc = psc[:, qi, 65 * h:65 * h + 65]
                        if kp == 0:
                            nc.tensor.matmul(
                                acc,
                                ep[32 * h:32 * h + PP,
                                   qi * 128:(qi + 1) * 128],
                                vp_sb[32 * h:32 * h + PP, h, :],
                                start=False, stop=False,
                                skip_group_check=True,
                            )
                        for i in range(2):
                            kt = 2 * kp + i
                            nc.tensor.matmul(
                                acc,
                                expt[:, i, qi * 128:(qi + 1) * 128],
                                v_sb[:, kt, h, :],
                                start=False, stop=False,
                                skip_group_check=True,
                            )

                # ---- norm + transpose + out-projection per sq-block ----
                outproj_work = []

                norm_work = []

                def emit_norm_qi(sqb, qi):
                    pscq = psc[:, qi, 0:195].rearrange(
                        "p (h c) -> p h c", c=65)
                    rc = rc_pool.tile([128, 4], f32, tag="rc", name="rc")
                    nc.vector.reciprocal(
                        rc[:, 0:HL], pscq[:, :, D])
                    for h in range(HL):
                        nc.vector.tensor_scalar_mul(
                            ctxn_sb[:, qi, h, :],
                            pscq[:, h, 0:D],
                            rc[:, h:h + 1])
                    qs = slice(sqb * SQB + qi * 128,
                               sqb * SQB + (qi + 1) * 128)
                    nc.sync.dma_start(
                        ctxT_sb[:, 0, qs], ctxn_sb[:, qi, 0:2, :],
                        transpose=True)
                    nc.sync.dma_start(
                        ctxT_sb[:, 1, qs], ctxn_sb[:, qi, 2:4, :],
                        transpose=True)
                    if qi == SQB // 128 - 1:
                        nc.vector.memset(psc[:], 0.0)
                        # delay the out-projection matmuls so their
                        # transpose dependencies clear before they enter
                        # PE's in-order queue
                        for et in range(6):
                            outproj_work.append(
                                (cur_t[0] + OP_DELAY + OP_SP * et, et, sqb))

                def emit_norm(sqb):
                    for qi in range(SQB // 128):
                        norm_work.append((cur_t[0] + qi, sqb, qi))

                def emit_outproj_tile(due, et, sqb):
                    es = slice(et * 128, (et + 1) * 128)
                    qs = slice(sqb * SQB, (sqb + 1) * SQB)
                    po3 = ps_str.tile([128, 512], f32, tag="u", name="po3")
                    for qi in range(SQB // 128):
                        ns = slice(qi * 128, (qi + 1) * 128)
                        qs2 = slice(sqb * SQB + qi * 128,
                                    sqb * SQB + (qi + 1) * 128)
                        nc.tensor.matmul(
                            po3[:, ns], wo_sb[:, 0, es], ctxT_sb[:, 0, qs2],
                            start=True, stop=False,
                        )
                        nc.tensor.matmul(
                            po3[:, ns], wo_sb[0:64, 1, es],
                            ctxT_sb[0:64, 1, qs2],
                            start=False, stop=True,
                        )
                    ot = out_pool.tile([128, 512], bf16, tag="ot", name="ot")
                    if OUT_DVE and sqb < NSQB - 1 and et % 2 == 0:
                        nc.vector.tensor_copy(ot[:], po3[:])
                    else:
                        nc.scalar.copy(ot[:], po3[:])
                    nc.gpsimd.dma_start(outT[es, qs], ot[:])

                # ---- the slot stream ----
                slots = [(sqb, kp, h)
                         for sqb in range(NSQB)
                         for kp in range(NKP)
                         for h in range(HL)]
                pending = []
                vst = 0
                cur_t = [0]
                hold_until = [INIT_HOLD]

                def pop_one():
                    (s2, e2, ep2) = pending.pop(0)
                    sqb2, kp2, h2 = s2
                    emit_ctx(sqb2, kp2, h2, e2, ep2)
                    if kp2 == NKP - 1 and h2 == HL - 1:
                        emit_norm(sqb2)
                        hold_until[0] = cur_t[0] + POP_HOLD
                        return True
                    return False

                for t, slot in enumerate(slots):
                    cur_t[0] = t
                    sqb, kp, h = slot
                    if kp == 0 and h == 0:
                        emit_prefix(sqb)
                    expt = emit_scores_exp(t, *slot)
                    pending.append((slot, expt, expp_cur[0]))
                    if vst < NST // 2 and t % VP_PACE == 0:
                        emit_vproj_pair(vst)
                        vst += 1
                    if t < 60:
                        trail_eff = TRAIL0
                    elif t < len(slots) - TAPER:
                        # per-block taper: drain the trail during each
                        # block's last slots (PE-light there) so the norm
                        # lands at the boundary, not 8 slots into the next
                        p = t % SLOTS_PER_SQB
                        if p >= TAPER_AT or p < TAPER_POST:
                            trail_eff = TRAIL_MIN
                        else:
                            trail_eff = TRAIL
                    else:
                        trail_eff = 1
                    if t >= hold_until[0]:
                        for _ in range(3):
                            if len(pending) > trail_eff:
                                # stop popping the moment a block ends: the
                                # next block's psc matmuls must not be
                                # emitted before the deferred norm + memset
                                if pop_one():
                                    break
                            else:
                                break
                    while norm_work and norm_work[0][0] <= t:
                        emit_norm_qi(*norm_work.pop(0)[1:])
                    while bg_work and bg_work[0][0] <= t:
                        bg_work.pop(0)[1]()
                    if outproj_work and outproj_work[0][0] <= t:
                        emit_outproj_tile(*outproj_work.pop(0))
                while pending:
                    pop_one()
                    while norm_work:
                        emit_norm_qi(*norm_work.pop(0)[1:])
                    if outproj_work:
                        emit_outproj_tile(*outproj_work.pop(0))
                while norm_work:
                    emit_norm_qi(*norm_work.pop(0)[1:])
                for _, op in bg_work:
                    op()
                while outproj_work:
                    emit_outproj_tile(*outproj_work.pop(0))

    nc.compile()
    return nc


def _get_nc():
    with _lock:
        if "nc" not in _compiled:
            _compiled["nc"] = _build()
        return _compiled["nc"]


def _prep_in_maps(query, key, value, prompt, Wq, bq, Wk, bk, Wv, bv, Wo, bo):
    f32 = np.float32
    qT = [np.ascontiguousarray(query[b].T).astype(BF16) for b in range(B)]
    kT = [np.ascontiguousarray(key[b].T).astype(BF16) for b in range(B)]
    vT = [np.ascontiguousarray(value[b].T).astype(BF16) for b in range(B)]
    in_maps = []
    for core in range(NCORES):
        b, g = core // NG, core % NG
        cs = slice(g * CL, (g + 1) * CL)
        kp = np.zeros((128, 2, PP), E4M3)
        vpa = np.zeros((128, HL, D + 1), BF16)
        for h in range(HL):
            gh = g * HL + h
            kp[64 * (h % 2):64 * (h % 2) + 64, h // 2, :] = (
                prompt[b, 0, :, gh, :].T.astype(E4M3))
            vpa[32 * h:32 * h + PP, h, D] = 1.0
            vpa[32 * h:32 * h + PP, h, 0:D] = (
                prompt[b, 1, :, gh, :].astype(BF16))
        in_maps.append({
            "xqT": qT[b], "xkT": kT[b], "xvT": vT[b],
            "wqT": np.ascontiguousarray(Wq[cs, :].T).astype(BF16),
            "wkT": np.ascontiguousarray(Wk[cs, :].T).astype(BF16),
            "wvT": np.ascontiguousarray(Wv[cs, :].T).astype(BF16),
            "woT": np.ascontiguousarray(Wo[:, cs].T).astype(BF16),
            "bq": np.ascontiguousarray(bq[cs]).astype(f32).reshape(CL, 1),
            "bk": np.ascontiguousarray(bk[cs]).astype(f32).reshape(CL, 1),
            "bv": np.ascontiguousarray(bv[cs]).astype(f32).reshape(1, CL),
            "kpT": kp, "vp": vpa,
        })
    return in_maps


def _combine(results, bo):
    out = np.empty((B, S, E), np.float32)
    for b in range(B):
        acc = results[b * NG]["outT"].astype(np.float32)
        for g in range(1, NG):
            acc = acc + results[b * NG + g]["outT"].astype(np.float32)
        out[b] = acc.T
    if bo is not None and np.any(bo):
        out += np.asarray(bo, np.float32)
    return out


def run(inputs, trace=False):
    """Returns (output, exec_time_ns or None)."""
    from concourse import bass_utils

    nc = _get_nc()
    in_maps = _prep_in_maps(**{k: np.asarray(v) for k, v in inputs.items()})
    bo = np.asarray(inputs["bo"])
    res = bass_utils.run_bass_kernel_spmd(
        nc, in_maps, core_ids=list(range(NCORES)), trace=trace,
    )
    return _combine(res.results, bo), res.exec_time_ns


def kernel(**inputs):
    out, _ = run(inputs)
    return out


# revision 45
# speedup vs baseline: 1.0121x; 1.0038x over previous
"""Multi-head attention (B=2,S=4096,E=768,H=12,D=64 + 16-token K/V prompt
prefix) on 8 Trainium2 NeuronCores.

Sharding: 2 batches x 4 head-groups (3 heads each). Each core computes QKV
projections for its 3 heads, full attention over its batch, and a partial
output projection (its 192 ctx channels); the host sums the 4 partials per
batch.

v3 design (413us -> 369us): the q/k activations are stored as fp8e4m3
and the scores matmuls run in DoubleRow perf mode (0.5 cycles per output
row -> 2x the bf16 scores throughput). DoubleRow needs a [K, 2, N]
two-k-tile operand shape but the scores contraction is only d=64, so both
j-planes read the SAME data via a stride-0 broadcast dim (psum = 2*kT^T q
exactly; the 0.5 folds into the exp scale). End-to-end rel err ~1.76e-2
(fp8 scores ~1.2e-2 + Schraudolph ~1.2e-2 + bf16 base ~0.45e-2, adding in
quadrature) vs the 2e-2 budget, measured in numpy and on device.

Scores for a kt-PAIR (2x128 kpos) land in one 2-bank psum tile and are
exp'd by a single [128, 1024] activation op, alternating ScalarE (exact
exp) / DVE (Schraudolph bit-trick, F_DVE of tiles) as in v2 but with half
the per-op overhead. ctx stays bf16 (fp8 expt/v would blow the error
budget). All other matmuls bf16. Slot stream is kt-pair-major: 8 sqb x 16
kp x 3 heads = 384 slots.

Per-core layout:
  qT8[d,2,s], kT8[d,2,s] fp8  ([128, pr, S]; DR via stride-0 broadcast)
  v[s,c]           natural + ones col  (denominator in ctx col 64)
  scoresT[k,q]     = DR(kT8-tile, qT8)  (psum [128, 2, 512] = kt-pair)
  expT             = Exp(scores/8)      ([128,1024]: ScalarE | DVE bit-trick)
  ctx[q,c+1]       = expT-tile^T @ v    (psc [128, 4, 256] bank-aligned,
                                         zeroed by first-touch start=True)
  ctx_norm         = ctx * recip(ctx[:,64])
  ctxT             = xbar DMA transpose of ctx_norm head-pairs
  outT[e,q]        = Wo-tiles^T @ ctxT  (partial; host sums 4 groups, bf16)
"""

import sys
import threading

import numpy as np

if "/opt/trn_rl_repo" not in sys.path:
    sys.path.insert(0, "/opt/trn_rl_repo")

import ml_dtypes

BF16 = ml_dtypes.bfloat16
E4M3 = ml_dtypes.float8_e4m3

B, S, E, H, D, PP = 2, 4096, 768, 12, 64, 16
NCORES = 8
NG = 4          # head-groups (tensor parallel)
HL = H // NG    # 3 local heads
CL = HL * D     # 192 local channels
NKT = S // 128  # 32 k-tiles
NKP = NKT // 2  # 16 kt-pairs
SQB = 512       # q block width in the attention stream
NSQB = S // SQB
NST = S // 128  # v s-tiles
QT = 1024       # q width for projection blocks
NQB = S // QT   # 1024-q projection blocks
SLOTS_PER_SQB = HL * NKP        # 48
TRAIL = 15      # ctx trails scores/exp by this many pair-slots
TRAIL0 = 15     # uniform (deep-trail corruption fixed by pop break)
F_DVE = 0.45    # mid-block DVE exp share (see f_dve_at)
F_SQB0 = 0.34   # DVE share in the PE-bound first block
F_LOW = 0.28    # DVE share in the post-boundary congestion window
WLO, WHI = 6, 18  # congestion window within a block
EXTRA_EXPT = 4
OP_DELAY = 12   # slots between norm emission and out-proj matmuls
EPI_DELAY = 2   # slots between bg-proj matmuls and their Act epilogue
POP_HOLD = 4    # MUST exceed the 0..3 norm spread: pops emit the next
                # block's psc matmuls, which must follow the qi3 memset
INIT_HOLD = 10  # first pop waits this many slots (vproj warm-up)
OUT_DVE = 1
VP_PACE = 3
TAPER = 4       # pop pause shrink at the end of the stream
TAPER_AT = 99   # per-block taper disabled (was neutral-to-harmful)
TAPER_POST = 0  # keep the low trail into the next block
QBG_OFF = -94   # bg q-proj placement relative to its deadline
OP_SP = 2       # out-proj tile spacing
KG_OFF = 9      # k-bg group position within its window
TRAIL_MIN = 3
LOG2E = 1.4426950408889634

_lock = threading.Lock()
_compiled = {}


def _build():
    import concourse.bass as bass  # noqa: F401
    import concourse.mybir as mybir
    import concourse.tile as tile
    from concourse import bacc

    f32 = mybir.dt.float32
    bf16 = mybir.dt.bfloat16
    fp8 = mybir.dt.float8e4
    i16 = mybir.dt.int16
    EXP = mybir.ActivationFunctionType.Exp
    MULT = mybir.AluOpType.mult
    ADD = mybir.AluOpType.add
    DR = mybir.MatmulPerfMode.DoubleRow

    nc = bacc.Bacc("TRN2", target_bir_lowering=False, debug=False)

    xqT = nc.dram_tensor("xqT", [E, S], bf16, kind="ExternalInput").ap()
    xkT = nc.dram_tensor("xkT", [E, S], bf16, kind="ExternalInput").ap()
    xvT = nc.dram_tensor("xvT", [E, S], bf16, kind="ExternalInput").ap()
    wqT = nc.dram_tensor("wqT", [E, CL], bf16, kind="ExternalInput").ap()
    wkT = nc.dram_tensor("wkT", [E, CL], bf16, kind="ExternalInput").ap()
    wvT = nc.dram_tensor("wvT", [E, CL], bf16, kind="ExternalInput").ap()
    woT = nc.dram_tensor("woT", [CL, E], bf16, kind="ExternalInput").ap()
    bq = nc.dram_tensor("bq", [CL, 1], f32, kind="ExternalInput").ap()
    bk = nc.dram_tensor("bk", [CL, 1], f32, kind="ExternalInput").ap()
    bv = nc.dram_tensor("bv", [1, CL], f32, kind="ExternalInput").ap()
    kpT = nc.dram_tensor("kpT", [128, 2, PP], fp8, kind="ExternalInput").ap()
    vp = nc.dram_tensor("vp", [128, HL, D + 1], bf16, kind="ExternalInput").ap()
    outT = nc.dram_tensor("outT", [E, S], bf16, kind="ExternalOutput").ap()

    # Schraudolph constants: psum holds raw q.k scores; exp arg = psum/8.
    # bf16 bits = 128*log2(e^(x)) + 16256 => psum * (128*log2e/8) + const.
    SCH_A = 128.0 * LOG2E * 0.0625  # doubled scores
    SCH_B = 16256.0 + 0.5 - 5.8  # +0.5 trunc->round, -5.8 sawtooth centering

    with tile.TileContext(nc) as tc:
        with tc.tile_pool(name="persist", bufs=1) as pers:
            # q-projection weights/bias first: they gate the very first
            # matmuls
            wq_sb = pers.tile([128, 6, CL], bf16)
            nc.gpsimd.dma_start(wq_sb[:], wqT.rearrange("(t p) c -> p t c", p=128))
            bq_sb = pers.tile([128, 2], f32)
            nc.gpsimd.dma_start(bq_sb[:, 0:1], bq[0:128, :])
            nc.gpsimd.dma_start(bq_sb[0:64, 1:2], bq[128:CL, :])

            wk_sb = pers.tile([128, 6, CL], bf16)
            wv_sb = pers.tile([128, 6, CL], bf16)
            wo_sb = pers.tile([128, 2, E], bf16)
            bk_sb = pers.tile([128, 2], f32)
            bvb_sb = pers.tile([128, CL], f32)
            kpT_sb = pers.tile([128, 2, PP], fp8)
            # prefix v: head h at partitions 32h..32h+PP (matches ep rows)
            vp_sb = pers.tile([128, HL, D + 1], bf16)

            # fp8 activations: [partition(d within head-pair), pr, S].
            # DoubleRow reads each operand TWICE via a stride-0 broadcast
            # j-dim, so psum = 2 * kT^T q exactly; the 0.5 folds into the
            # exp scale (0.0625 instead of 0.125).
            qT8_sb = pers.tile([128, 2, S], fp8)
            kT8_sb = pers.tile([128, 2, S], fp8)
            v_sb = pers.tile([128, NST, HL, D + 1], bf16)
            ctxT_sb = pers.tile([128, 2, S], bf16)
            # ctx_norm staging for xbar transposes: [qi, (h0,h1,h2,pad), d]
            ctxn_sb = pers.tile([128, SQB // 128, 4, D], bf16)

            nc.vector.memset(v_sb[:, :, :, D:D + 1], 1.0)
            nc.vector.memset(ctxn_sb[:, :, 3, :], 0.0)

            # ---------------- Phase 1a: Q / K projections ----------------
            with (
                tc.tile_pool(name="ps_proj", bufs=2, space="PSUM") as pp,
                tc.tile_pool(name="xq_pool", bufs=12) as xq_pool,
            ):
                # k/prefix weights on the Pool queue while ALL 12 q+k
                # input chunks stream interleaved on the faster SP/HWDGE
                # queue; non-critical weights (wv/wo/prefix-v) follow on
                # Pool after
                nc.gpsimd.dma_start(
                    wk_sb[:], wkT.rearrange("(t p) c -> p t c", p=128))
                nc.gpsimd.dma_start(bk_sb[:, 0:1], bk[0:128, :])
                nc.gpsimd.dma_start(bk_sb[0:64, 1:2], bk[128:CL, :])
                nc.gpsimd.dma_start(kpT_sb[:], kpT[:])

                # ALL 12 q+k input chunks stream interleaved on the
                # faster SP/HWDGE queue
                xts = {}
                for ech in range(6):
                    for which, xin in (("q", xqT), ("k", xkT)):
                        xt = xq_pool.tile([128, QT], bf16, tag="xt",
                                          name="xt")
                        nc.sync.dma_start(
                            xt[:], xin[ech * 128:(ech + 1) * 128, 0:QT])
                        xts[(which, ech)] = xt

                def proj_block0(which, wsb, bsb, dst):
                    p0 = pp.tile([128, QT], f32, tag="p0", name="p0")
                    p1 = pp.tile([64, QT], f32, tag="p1", name="p1")
                    for ech in range(6):
                        xt = xts[(which, ech)]
                        for n in range(QT // 512):
                            ns = slice(n * 512, (n + 1) * 512)
                            nc.tensor.matmul(
                                p0[:, ns], wsb[:, ech, 0:128], xt[:, ns],
                                start=(ech == 0), stop=(ech == 5),
                            )
                            nc.tensor.matmul(
                                p1[:, ns], wsb[:, ech, 128:CL], xt[:, ns],
                                start=(ech == 0), stop=(ech == 5),
                            )
                    for n in range(QT // 512):
                        ds = slice(n * 512, (n + 1) * 512)
                        ns = slice(n * 512, (n + 1) * 512)
                        nc.scalar.add(dst[:, 0, ds], p0[:, ns], bsb[:, 0:1])
                        nc.scalar.add(dst[0:64, 1, ds], p1[0:64, ns],
                                      bsb[0:64, 1:2])

                proj_block0("q", wq_sb, bq_sb, qT8_sb)
                proj_block0("k", wk_sb, bk_sb, kT8_sb)
                nc.gpsimd.dma_start(
                    wv_sb[:], wvT.rearrange("(t p) c -> p t c", p=128))
                nc.gpsimd.dma_start(bvb_sb[:], bv.to_broadcast((128, CL)))
                nc.gpsimd.dma_start(vp_sb[:], vp[:])
                nc.gpsimd.dma_start(wo_sb[:, 0, :], woT[0:128, :])
                nc.gpsimd.dma_start(wo_sb[0:64, 1, :], woT[128:CL, :])

            # ---------- attention stream ----------
            # slot order is kp-major: for each sq-block, sweep kt-pairs in
            # order with the 3 heads innermost. Background k/q projections
            # and the v-projection are spread across the stream so the DMA
            # engines never burst.
            # One unified 3-deep psum ring (tag "u", 2-bank slots = 6 banks)
            # serves scores pairs AND the proj/vproj/prefix/outproj scratch;
            # psc takes the other 2 banks. Depth 3 hides the
            # matmul->exp->psum-free turnaround that a 2-deep ring cannot.
            with (
                tc.tile_pool(name="ps_str", bufs=3, space="PSUM") as ps_str,
                tc.tile_pool(name="ps_acc", bufs=1, space="PSUM") as ps_acc,
                tc.tile_pool(name="expt_pool",
                             bufs=max(TRAIL, TRAIL0) + POP_HOLD
                             + EXTRA_EXPT) as expt_pool,
                tc.tile_pool(name="expp_pool", bufs=2) as expp_pool,
                tc.tile_pool(name="rc_pool", bufs=4) as rc_pool,
                tc.tile_pool(name="xv_pool", bufs=8) as xv_pool,
                tc.tile_pool(name="xq2_pool", bufs=7) as xq2_pool,
                tc.tile_pool(name="xk2_pool", bufs=12) as xk2_pool,
                tc.tile_pool(name="out_pool", bufs=4) as out_pool,
            ):
                # ctx accumulator: 2 banks; slice (qi, h) = [128q, 65].
                # 12 interleaved accumulation chains share the banks, so
                # matmul start=True (bank-granular zeroing) cannot be used:
                # the tile is DVE-memset per sq-block and every ctx matmul
                # accumulates with start=False.
                # [128, qi, 256]: each qi stride 1KB, 2 qi per psum bank.
                # Explicitly memset between blocks (on DVE, in-order after
                # the norm reads) -- matmul start=True bank-zeroing would
                # race with the deferred norm reads of the previous block.
                psc = ps_acc.tile([128, SQB // 128, 256], f32, name="psc")
                nc.vector.memset(psc[:], 0.0)

                # ---- background projections, chunk-granular ----
                # each (c, grp) group is split into a matmul op and a
                # deferred Act epilogue so the Act in-order queue never
                # parks on a bias-add whose matmuls are still running
                def make_bg_proj(xin, wsb, bsb, dst, sq, dma_eng, pool):
                    mm_ops = []
                    epi_ops = []
                    state = {}

                    def mk_dma(ech):
                        def op():
                            xt2 = pool.tile([128, QT], bf16, tag="xt2",
                                            name="xt2")
                            dma_eng.dma_start(
                                xt2[:],
                                xin[ech * 128:(ech + 1) * 128,
                                    sq * QT:(sq + 1) * QT],
                            )
                            state[ech] = xt2
                        return op

                    def mk_group(c, grp):
                        def mm_op():
                            pt = ps_str.tile([128, 512], f32, tag="u",
                                             name="pq")
                            rows = 128 if grp == 0 else 64
                            wc = slice(0, 128) if grp == 0 else slice(128, CL)
                            for ech in range(6):
                                nc.tensor.matmul(
                                    pt[0:rows, :], wsb[:, ech, wc],
                                    state[ech][:, c * 512:(c + 1) * 512],
                                    start=(ech == 0), stop=(ech == 5),
                                )
                            state[("pt", c, grp)] = pt

                        def epi_op():
                            pt = state.pop(("pt", c, grp))
                            qs = slice(sq * QT + c * 512,
                                       sq * QT + (c + 1) * 512)
                            if grp == 0:
                                nc.scalar.add(
                                    dst[:, 0, qs], pt[:, :], bsb[:, 0:1])
                            else:
                                nc.scalar.add(
                                    dst[0:64, 1, qs], pt[0:64, :],
                                    bsb[0:64, 1:2])
                        return mm_op, epi_op

                    for ech in range(6):
                        mm_ops.append(mk_dma(ech))
                    for c in range(QT // 512):
                        for grp in range(2):
                            mm, epi = mk_group(c, grp)
                            mm_ops.append(mm)
                            epi_ops.append(epi)
                    return mm_ops, epi_ops

                bg_work = []
                # k blocks 1..: block b first needed at pair-slot 12b
                for sq in range(1, NQB):
                    base = 12 * (sq - 1)
                    mm_ops, epi_ops = make_bg_proj(
                        xkT, wk_sb, bk_sb, kT8_sb, sq, nc.sync, xk2_pool)
                    tags = [base + i for i in range(6)] + \
                           [base + KG_OFF + i for i in range(4)]
                    for tg, op in zip(tags, mm_ops):
                        bg_work.append((tg, op))
                    for i, op in enumerate(epi_ops):
                        bg_work.append((base + KG_OFF + i + EPI_DELAY, op))
                # q blocks 1..: block n needed by pair-slot 96n
                for sq in range(1, NQB):
                    t0 = max(14, 2 * sq * SLOTS_PER_SQB + QBG_OFF)
                    mm_ops, epi_ops = make_bg_proj(
                        xqT, wq_sb, bq_sb, qT8_sb, sq, nc.gpsimd, xq2_pool)
                    for i, op in enumerate(mm_ops):
                        bg_work.append((t0 + i, op))
                    for i, op in enumerate(epi_ops):
                        bg_work.append((t0 + 6 + i + EPI_DELAY, op))
                bg_work.sort(key=lambda x: x[0])

                # ---- V projection (natural orientation) ----
                xvts = {}

                def load_xv_chunk(sqx, ech, eng=None):
                    xvt = xv_pool.tile([128, QT], bf16, tag="xvt",
                                       name="xvt")
                    (eng or nc.gpsimd).dma_start(
                        xvt[:],
                        xvT[ech * 128:(ech + 1) * 128,
                            sqx * QT:(sqx + 1) * QT],
                    )
                    xvts[(sqx, ech)] = xvt

                def emit_vproj_pair(stp):
                    # two s-tiles (2*stp, 2*stp+1) share one psum tile
                    pv = ps_str.tile([128, 2, 512], f32, tag="u", name="pv")
                    for half in range(2):
                        st = 2 * stp + half
                        sqx, stl = st // (QT // 128), st % (QT // 128)
                        if st == 0:
                            for ech in range(6):
                                load_xv_chunk(0, ech, nc.sync)
                        if stl < 6 and sqx + 1 < NQB:
                            load_xv_chunk(sqx + 1, stl)
                        for ech in range(6):
                            nc.tensor.matmul(
                                pv[:, half, 0:CL],
                                xvts[(sqx, ech)][:,
                                                 stl * 128:(stl + 1) * 128],
                                wv_sb[:, ech, :],
                                start=(ech == 0), stop=(ech == 5),
                            )
                    for half in range(2):
                        st = 2 * stp + half
                        sqx, stl = st // (QT // 128), st % (QT // 128)
                        nc.vector.tensor_add(
                            v_sb[:, st, :, 0:D],
                            pv[:, half, 0:CL].rearrange(
                                "p (h d) -> p h d", h=HL),
                            bvb_sb[:].rearrange("p (h d) -> p h d", h=HL),
                        )
                        if stl == (QT // 128) - 1:
                            for ech in range(6):
                                del xvts[(sqx, ech)]

                # ---- prefix scores + exp for one sq-block (3 heads) ----
                expp_cur = [None]

                def emit_prefix(sqb):
                    psm = ps_str.tile([128, 512], f32, tag="u", name="psp")
                    qs = slice(sqb * SQB, (sqb + 1) * SQB)
                    # head h lives at partitions 32h..32h+16 (same rows as
                    # its scores psum), so one [80, 512] activation handles
                    # all three heads; rows 16-31/48-63 are junk but finite
                    ep = expp_pool.tile([128, SQB], bf16, tag="ep",
                                        name="ep")
                    for h in range(HL):
                        pr, po = h // 2, 64 * (h % 2)
                        nc.tensor.matmul(
                            psm[32 * h:32 * h + PP, :],
                            kpT_sb[po:po + 64, pr, :],
                            qT8_sb[po:po + 64, pr, qs],
                            start=True, stop=True,
                        )
                    if sqb == 0:
                        # ring psum is uninitialized at t=0: only touch the
                        # rows the matmuls wrote
                        for h in range(HL):
                            nc.scalar.activation(
                                ep[32 * h:32 * h + PP, :],
                                psm[32 * h:32 * h + PP, :],
                                EXP, scale=0.125)
                    else:
                        nc.scalar.activation(
                            ep[0:80, :], psm[0:80, :], EXP, scale=0.125)
                    expp_cur[0] = ep

                # ---- scores (DoubleRow fp8) + exp for one (sqb, kp, h) ----
                # dynamic Schraudolph share: Act absorbs more exp where DVE
                # is congested (right after a block's norm) and in the
                # PE-bound first block; DVE takes more mid-block
                dve_acc = [0.0]

                def f_dve_at(t):
                    if t < SLOTS_PER_SQB + 6:
                        return F_SQB0
                    p = t % SLOTS_PER_SQB
                    if WLO <= p < WHI:
                        return F_LOW
                    return F_DVE

                def emit_scores_exp(t, sqb, kp, h):
                    pr, po = h // 2, 64 * (h % 2)
                    qs = slice(sqb * SQB, (sqb + 1) * SQB)
                    pss = ps_str.tile([128, 2, 512], f32, tag="u",
                                      name="pss")
                    qmov = qT8_sb[po:po + 64, pr:pr + 1, qs].to_broadcast(
                        (64, 2, SQB))
                    for i in range(2):
                        kt = 2 * kp + i
                        nc.tensor.matmul(
                            pss[:, i, :],
                            kT8_sb[po:po + 64, pr:pr + 1,
                                   kt * 128:(kt + 1) * 128].to_broadcast(
                                (64, 2, 128)),
                            qmov,
                            start=True, stop=True,
                            perf_mode=DR,
                        )
                    expt = expt_pool.tile([128, 2, 512], bf16, tag="expt",
                                          name="expt")
                    # near-alternating engine assignment so consecutive exps
                    # overlap across ScalarE / DVE
                    use_dve = False
                    dve_acc[0] += f_dve_at(t)
                    if dve_acc[0] >= 1.0:
                        dve_acc[0] -= 1.0
                        use_dve = True
                    if use_dve:
                        nc.vector.tensor_scalar(
                            expt[:].bitcast(i16), pss[:], SCH_A, SCH_B,
                            MULT, ADD)
                    else:
                        nc.scalar.activation(expt[:], pss[:], EXP,
                                             scale=0.0625)
                    return expt

                # ---- ctx (natural orientation) ----
                def emit_ctx(sqb, kp, h, expt, ep):
                    for qi in range(SQB // 128):
                        acc = psc[:, qi, 65 * h:65 * h + 65]
                        if kp == 0:
                            nc.tensor.matmul(
                                acc,
                                ep[32 * h:32 * h + PP,
                                   qi * 128:(qi + 1) * 128],
                                vp_sb[32 * h:32 * h + PP, h, :],
                                start=False, stop=False,
                                skip_group_check=True,
                            )
                        for i in range(2):
                            kt = 2 * kp + i
                            nc.tensor.matmul(
                                acc,
                                expt[:, i, qi * 128:(qi + 1) * 128],
                                v_sb[:, kt, h, :],
                                start=False, stop=False,
                                skip_group_check=True,
                            )

                # ---- norm + transpose + out-projection per sq-block ----
                outproj_work = []

                norm_work = []

                def emit_norm_qi(sqb, qi):
                    pscq = psc[:, qi, 0:195].rearrange(
                        "p (h c) -> p h c", c=65)
                    rc = rc_pool.tile([128, 4], f32, tag="rc", name="rc")
                    nc.vector.reciprocal(
                        rc[:, 0:HL], pscq[:, :, D])
                    for h in range(HL):
                        nc.vector.tensor_scalar_mul(
                            ctxn_sb[:, qi, h, :],
                            pscq[:, h, 0:D],
                            rc[:, h:h + 1])
                    qs = slice(sqb * SQB + qi * 128,
                               sqb * SQB + (qi + 1) * 128)
                    nc.sync.dma_start(
                        ctxT_sb[:, 0, qs], ctxn_sb[:, qi, 0:2, :],
                        transpose=True)
                    nc.sync.dma_start(
                        ctxT_sb[:, 1, qs], ctxn_sb[:, qi, 2:4, :],
                        transpose=True)
                    if qi == SQB // 128 - 1:
                        nc.vector.memset(psc[:], 0.0)
                        # delay the out-projection matmuls so their
                        # transpose dependencies clear before they enter
                        # PE's in-order queue
                        for et in range(6):
                            outproj_work.append(
                                (cur_t[0] + OP_DELAY + OP_SP * et, et, sqb))

                def emit_norm(sqb):
                    for qi in range(SQB // 128):
                        norm_work.append((cur_t[0] + qi, sqb, qi))

                def emit_outproj_tile(due, et, sqb):
                    es = slice(et * 128, (et + 1) * 128)
                    qs = slice(sqb * SQB, (sqb + 1) * SQB)
                    po3 = ps_str.tile([128, 512], f32, tag="u", name="po3")
                    for qi in range(SQB // 128):
                        ns = slice(qi * 128, (qi + 1) * 128)
                        qs2 = slice(sqb * SQB + qi * 128,
                                    sqb * SQB + (qi + 1) * 128)
                        nc.tensor.matmul(
                            po3[:, ns], wo_sb[:, 0, es], ctxT_sb[:, 0, qs2],
                            start=True, stop=False,
                        )
                        nc.tensor.matmul(
                            po3[:, ns], wo_sb[0:64, 1, es],
                            ctxT_sb[0:64, 1, qs2],
                            start=False, stop=True,
                        )
                    ot = out_pool.tile([128, 512], bf16, tag="ot", name="ot")
                    if OUT_DVE and sqb < NSQB - 1 and et % 2 == 0:
                        nc.vector.tensor_copy(ot[:], po3[:])
                    else:
                        nc.scalar.copy(ot[:], po3[:])
                    nc.gpsimd.dma_start(outT[es, qs], ot[:])

                # ---- the slot stream ----
                slots = [(sqb, kp, h)
                         for sqb in range(NSQB)
                         for kp in range(NKP)
                         for h in range(HL)]
                pending = []
                vst = 0
                cur_t = [0]
                hold_until = [INIT_HOLD]

                def pop_one():
                    (s2, e2, ep2) = pending.pop(0)
                    sqb2, kp2, h2 = s2
                    emit_ctx(sqb2, kp2, h2, e2, ep2)
                    if kp2 == NKP - 1 and h2 == HL - 1:
                        emit_norm(sqb2)
                        hold_until[0] = cur_t[0] + POP_HOLD
                        return True
                    return False

                for t, slot in enumerate(slots):
                    cur_t[0] = t
                    sqb, kp, h = slot
                    if kp == 0 and h == 0:
                        emit_prefix(sqb)
                    expt = emit_scores_exp(t, *slot)
                    pending.append((slot, expt, expp_cur[0]))
                    if vst < NST // 2 and t % VP_PACE == 0:
                        emit_vproj_pair(vst)
                        vst += 1
                    if t < 60:
                        trail_eff = TRAIL0
                    elif t < len(slots) - TAPER:
                        # per-block taper: drain the trail during each
                        # block's last slots (PE-light there) so the norm
                        # lands at the boundary, not 8 slots into the next
                        p = t % SLOTS_PER_SQB
                        if p >= TAPER_AT or p < TAPER_POST:
                            trail_eff = TRAIL_MIN
                        else:
                            trail_eff = TRAIL
                    else:
                        trail_eff = 1
                    if t >= hold_until[0]:
                        for _ in range(3):
                            if len(pending) > trail_eff:
                                # stop popping the moment a block ends: the
                                # next block's psc matmuls must not be
                                # emitted before the deferred norm + memset
                                if pop_one():
                                    break
                            else:
                                break
                    while norm_work and norm_work[0][0] <= t:
                        emit_norm_qi(*norm_work.pop(0)[1:])
                    while bg_work and bg_work[0][0] <= t:
                        bg_work.pop(0)[1]()
                    if outproj_work and outproj_work[0][0] <= t:
                        emit_outproj_tile(*outproj_work.pop(0))
                while pending:
                    pop_one()
                    while norm_work:
                        emit_norm_qi(*norm_work.pop(0)[1:])
                    if outproj_work:
                        emit_outproj_tile(*outproj_work.pop(0))
                while norm_work:
                    emit_norm_qi(*norm_work.pop(0)[1:])
                for _, op in bg_work:
                    op()
                while outproj_work:
                    emit_outproj_tile(*outproj_work.pop(0))

    nc.compile()
    return nc


def _get_nc():
    with _lock:
        if "nc" not in _compiled:
            _compiled["nc"] = _build()
        return _compiled["nc"]


def _prep_in_maps(query, key, value, prompt, Wq, bq, Wk, bk, Wv, bv, Wo, bo):
    f32 = np.float32
    qT = [np.ascontiguousarray(query[b].T).astype(BF16) for b in range(B)]
    kT = [np.ascontiguousarray(key[b].T).astype(BF16) for b in range(B)]
    vT = [np.ascontiguousarray(value[b].T).astype(BF16) for b in range(B)]
    in_maps = []
    for core in range(NCORES):
        b, g = core // NG, core % NG
        cs = slice(g * CL, (g + 1) * CL)
        kp = np.zeros((128, 2, PP), E4M3)
        vpa = np.zeros((128, HL, D + 1), BF16)
        for h in range(HL):
            gh = g * HL + h
            kp[64 * (h % 2):64 * (h % 2) + 64, h // 2, :] = (
                prompt[b, 0, :, gh, :].T.astype(E4M3))
            vpa[32 * h:32 * h + PP, h, D] = 1.0
            vpa[32 * h:32 * h + PP, h, 0:D] = (
                prompt[b, 1, :, gh, :].astype(BF16))
        in_maps.append({
            "xqT": qT[b], "xkT": kT[b], "xvT": vT[b],
            "wqT": np.ascontiguousarray(Wq[cs, :].T).astype(BF16),
            "wkT": np.ascontiguousarray(Wk[cs, :].T).astype(BF16),
            "wvT": np.ascontiguousarray(Wv[cs, :].T).astype(BF16),
            "woT": np.ascontiguousarray(Wo[:, cs].T).astype(BF16),
            "bq": np.ascontiguousarray(bq[cs]).astype(f32).reshape(CL, 1),
            "bk": np.ascontiguousarray(bk[cs]).astype(f32).reshape(CL, 1),
            "bv": np.ascontiguousarray(bv[cs]).astype(f32).reshape(1, CL),
            "kpT": kp, "vp": vpa,
        })
    return in_maps


def _combine(results, bo):
    out = np.empty((B, S, E), np.float32)
    for b in range(B):
        acc = results[b * NG]["outT"].astype(np.float32)
        for g in range(1, NG):
            acc = acc + results[b * NG + g]["outT"].astype(np.float32)
        out[b] = acc.T
    if bo is not None and np.any(bo):
        out += np.asarray(bo, np.float32)
    return out


def run(inputs, trace=False):
    """Returns (output, exec_time_ns or None)."""
    from concourse import bass_utils

    nc = _get_nc()
    in_maps = _prep_in_maps(**{k: np.asarray(v) for k, v in inputs.items()})
    bo = np.asarray(inputs["bo"])
    res = bass_utils.run_bass_kernel_spmd(
        nc, in_maps, core_ids=list(range(NCORES)), trace=trace,
    )
    return _combine(res.results, bo), res.exec_time_ns


def kernel(**inputs):
    out, _ = run(inputs)
    return out


# revision 46
# speedup vs baseline: 1.0157x; 1.0036x over previous
"""Multi-head attention (B=2,S=4096,E=768,H=12,D=64 + 16-token K/V prompt
prefix) on 8 Trainium2 NeuronCores.

Sharding: 2 batches x 4 head-groups (3 heads each). Each core computes QKV
projections for its 3 heads, full attention over its batch, and a partial
output projection (its 192 ctx channels); the host sums the 4 partials per
batch.

v3 design (413us -> 369us): the q/k activations are stored as fp8e4m3
and the scores matmuls run in DoubleRow perf mode (0.5 cycles per output
row -> 2x the bf16 scores throughput). DoubleRow needs a [K, 2, N]
two-k-tile operand shape but the scores contraction is only d=64, so both
j-planes read the SAME data via a stride-0 broadcast dim (psum = 2*kT^T q
exactly; the 0.5 folds into the exp scale). End-to-end rel err ~1.76e-2
(fp8 scores ~1.2e-2 + Schraudolph ~1.2e-2 + bf16 base ~0.45e-2, adding in
quadrature) vs the 2e-2 budget, measured in numpy and on device.

Scores for a kt-PAIR (2x128 kpos) land in one 2-bank psum tile and are
exp'd by a single [128, 1024] activation op, alternating ScalarE (exact
exp) / DVE (Schraudolph bit-trick, F_DVE of tiles) as in v2 but with half
the per-op overhead. ctx stays bf16 (fp8 expt/v would blow the error
budget). All other matmuls bf16. Slot stream is kt-pair-major: 8 sqb x 16
kp x 3 heads = 384 slots.

Per-core layout:
  qT8[d,2,s], kT8[d,2,s] fp8  ([128, pr, S]; DR via stride-0 broadcast)
  v[s,c]           natural + ones col  (denominator in ctx col 64)
  scoresT[k,q]     = DR(kT8-tile, qT8)  (psum [128, 2, 512] = kt-pair)
  expT             = Exp(scores/8)      ([128,1024]: ScalarE | DVE bit-trick)
  ctx[q,c+1]       = expT-tile^T @ v    (psc [128, 4, 256] bank-aligned,
                                         zeroed by first-touch start=True)
  ctx_norm         = ctx * recip(ctx[:,64])
  ctxT             = xbar DMA transpose of ctx_norm head-pairs
  outT[e,q]        = Wo-tiles^T @ ctxT  (partial; host sums 4 groups, bf16)
"""

import sys
import threading

import numpy as np

if "/opt/trn_rl_repo" not in sys.path:
    sys.path.insert(0, "/opt/trn_rl_repo")

import ml_dtypes

BF16 = ml_dtypes.bfloat16
E4M3 = ml_dtypes.float8_e4m3

B, S, E, H, D, PP = 2, 4096, 768, 12, 64, 16
NCORES = 8
NG = 4          # head-groups (tensor parallel)
HL = H // NG    # 3 local heads
CL = HL * D     # 192 local channels
NKT = S // 128  # 32 k-tiles
NKP = NKT // 2  # 16 kt-pairs
SQB = 512       # q block width in the attention stream
NSQB = S // SQB
NST = S // 128  # v s-tiles
QT = 1024       # q width for projection blocks
NQB = S // QT   # 1024-q projection blocks
SLOTS_PER_SQB = HL * NKP        # 48
TRAIL = 15      # ctx trails scores/exp by this many pair-slots
TRAIL0 = 15     # uniform (deep-trail corruption fixed by pop break)
F_DVE = 0.45    # mid-block DVE exp share (see f_dve_at)
F_SQB0 = 0.34   # DVE share in the PE-bound first block
F_LOW = 0.28    # DVE share in the post-boundary congestion window
WLO, WHI = 6, 18  # congestion window within a block
EXTRA_EXPT = 4
OP_DELAY = 12   # slots between norm emission and out-proj matmuls
EPI_DELAY = 2   # slots between bg-proj matmuls and their Act epilogue
POP_HOLD = 4    # MUST exceed the 0..3 norm spread: pops emit the next
                # block's psc matmuls, which must follow the qi3 memset
INIT_HOLD = 10  # first pop waits this many slots (vproj warm-up)
OUT_DVE = 1
VP_PACE = 3
TAPER = 4       # pop pause shrink at the end of the stream
TAPER_AT = 99   # per-block taper disabled (was neutral-to-harmful)
TAPER_POST = 0  # keep the low trail into the next block
QBG_OFF = -94   # bg q-proj placement relative to its deadline
OP_SP = 2       # out-proj tile spacing
KG_OFF = 8      # k-bg group position within its window
TRAIL_MIN = 3
LOG2E = 1.4426950408889634

_lock = threading.Lock()
_compiled = {}


def _build():
    import concourse.bass as bass  # noqa: F401
    import concourse.mybir as mybir
    import concourse.tile as tile
    from concourse import bacc

    f32 = mybir.dt.float32
    bf16 = mybir.dt.bfloat16
    fp8 = mybir.dt.float8e4
    i16 = mybir.dt.int16
    EXP = mybir.ActivationFunctionType.Exp
    MULT = mybir.AluOpType.mult
    ADD = mybir.AluOpType.add
    DR = mybir.MatmulPerfMode.DoubleRow

    nc = bacc.Bacc("TRN2", target_bir_lowering=False, debug=False)

    xqT = nc.dram_tensor("xqT", [E, S], bf16, kind="ExternalInput").ap()
    xkT = nc.dram_tensor("xkT", [E, S], bf16, kind="ExternalInput").ap()
    xvT = nc.dram_tensor("xvT", [E, S], bf16, kind="ExternalInput").ap()
    wqT = nc.dram_tensor("wqT", [E, CL], bf16, kind="ExternalInput").ap()
    wkT = nc.dram_tensor("wkT", [E, CL], bf16, kind="ExternalInput").ap()
    wvT = nc.dram_tensor("wvT", [E, CL], bf16, kind="ExternalInput").ap()
    woT = nc.dram_tensor("woT", [CL, E], bf16, kind="ExternalInput").ap()
    bq = nc.dram_tensor("bq", [CL, 1], f32, kind="ExternalInput").ap()
    bk = nc.dram_tensor("bk", [CL, 1], f32, kind="ExternalInput").ap()
    bv = nc.dram_tensor("bv", [1, CL], f32, kind="ExternalInput").ap()
    kpT = nc.dram_tensor("kpT", [128, 2, PP], fp8, kind="ExternalInput").ap()
    vp = nc.dram_tensor("vp", [128, HL, D + 1], bf16, kind="ExternalInput").ap()
    outT = nc.dram_tensor("outT", [E, S], bf16, kind="ExternalOutput").ap()

    # Schraudolph constants: psum holds raw q.k scores; exp arg = psum/8.
    # bf16 bits = 128*log2(e^(x)) + 16256 => psum * (128*log2e/8) + const.
    SCH_A = 128.0 * LOG2E * 0.0625  # doubled scores
    SCH_B = 16256.0 + 0.5 - 5.8  # +0.5 trunc->round, -5.8 sawtooth centering

    with tile.TileContext(nc) as tc:
        with tc.tile_pool(name="persist", bufs=1) as pers:
            # q-projection weights/bias first: they gate the very first
            # matmuls
            wq_sb = pers.tile([128, 6, CL], bf16)
            nc.gpsimd.dma_start(wq_sb[:], wqT.rearrange("(t p) c -> p t c", p=128))
            bq_sb = pers.tile([128, 2], f32)
            nc.gpsimd.dma_start(bq_sb[:, 0:1], bq[0:128, :])
            nc.gpsimd.dma_start(bq_sb[0:64, 1:2], bq[128:CL, :])

            wk_sb = pers.tile([128, 6, CL], bf16)
            wv_sb = pers.tile([128, 6, CL], bf16)
            wo_sb = pers.tile([128, 2, E], bf16)
            bk_sb = pers.tile([128, 2], f32)
            bvb_sb = pers.tile([128, CL], f32)
            kpT_sb = pers.tile([128, 2, PP], fp8)
            # prefix v: head h at partitions 32h..32h+PP (matches ep rows)
            vp_sb = pers.tile([128, HL, D + 1], bf16)

            # fp8 activations: [partition(d within head-pair), pr, S].
            # DoubleRow reads each operand TWICE via a stride-0 broadcast
            # j-dim, so psum = 2 * kT^T q exactly; the 0.5 folds into the
            # exp scale (0.0625 instead of 0.125).
            qT8_sb = pers.tile([128, 2, S], fp8)
            kT8_sb = pers.tile([128, 2, S], fp8)
            v_sb = pers.tile([128, NST, HL, D + 1], bf16)
            ctxT_sb = pers.tile([128, 2, S], bf16)
            # ctx_norm staging for xbar transposes: [qi, (h0,h1,h2,pad), d]
            ctxn_sb = pers.tile([128, SQB // 128, 4, D], bf16)

            nc.vector.memset(v_sb[:, :, :, D:D + 1], 1.0)
            nc.vector.memset(ctxn_sb[:, :, 3, :], 0.0)

            # ---------------- Phase 1a: Q / K projections ----------------
            with (
                tc.tile_pool(name="ps_proj", bufs=2, space="PSUM") as pp,
                tc.tile_pool(name="xq_pool", bufs=12) as xq_pool,
            ):
                # k/prefix weights on the Pool queue while ALL 12 q+k
                # input chunks stream interleaved on the faster SP/HWDGE
                # queue; non-critical weights (wv/wo/prefix-v) follow on
                # Pool after
                nc.gpsimd.dma_start(
                    wk_sb[:], wkT.rearrange("(t p) c -> p t c", p=128))
                nc.gpsimd.dma_start(bk_sb[:, 0:1], bk[0:128, :])
                nc.gpsimd.dma_start(bk_sb[0:64, 1:2], bk[128:CL, :])
                nc.gpsimd.dma_start(kpT_sb[:], kpT[:])

                # ALL 12 q+k input chunks stream interleaved on the
                # faster SP/HWDGE queue
                xts = {}
                for ech in range(6):
                    for which, xin in (("q", xqT), ("k", xkT)):
                        xt = xq_pool.tile([128, QT], bf16, tag="xt",
                                          name="xt")
                        nc.sync.dma_start(
                            xt[:], xin[ech * 128:(ech + 1) * 128, 0:QT])
                        xts[(which, ech)] = xt

                def proj_block0(which, wsb, bsb, dst):
                    p0 = pp.tile([128, QT], f32, tag="p0", name="p0")
                    p1 = pp.tile([64, QT], f32, tag="p1", name="p1")
                    for ech in range(6):
                        xt = xts[(which, ech)]
                        for n in range(QT // 512):
                            ns = slice(n * 512, (n + 1) * 512)
                            nc.tensor.matmul(
                                p0[:, ns], wsb[:, ech, 0:128], xt[:, ns],
                                start=(ech == 0), stop=(ech == 5),
                            )
                            nc.tensor.matmul(
                                p1[:, ns], wsb[:, ech, 128:CL], xt[:, ns],
                                start=(ech == 0), stop=(ech == 5),
                            )
                    for n in range(QT // 512):
                        ds = slice(n * 512, (n + 1) * 512)
                        ns = slice(n * 512, (n + 1) * 512)
                        nc.scalar.add(dst[:, 0, ds], p0[:, ns], bsb[:, 0:1])
                        nc.scalar.add(dst[0:64, 1, ds], p1[0:64, ns],
                                      bsb[0:64, 1:2])

                proj_block0("q", wq_sb, bq_sb, qT8_sb)
                proj_block0("k", wk_sb, bk_sb, kT8_sb)
                nc.gpsimd.dma_start(
                    wv_sb[:], wvT.rearrange("(t p) c -> p t c", p=128))
                nc.gpsimd.dma_start(bvb_sb[:], bv.to_broadcast((128, CL)))
                nc.gpsimd.dma_start(vp_sb[:], vp[:])
                nc.gpsimd.dma_start(wo_sb[:, 0, :], woT[0:128, :])
                nc.gpsimd.dma_start(wo_sb[0:64, 1, :], woT[128:CL, :])

            # ---------- attention stream ----------
            # slot order is kp-major: for each sq-block, sweep kt-pairs in
            # order with the 3 heads innermost. Background k/q projections
            # and the v-projection are spread across the stream so the DMA
            # engines never burst.
            # One unified 3-deep psum ring (tag "u", 2-bank slots = 6 banks)
            # serves scores pairs AND the proj/vproj/prefix/outproj scratch;
            # psc takes the other 2 banks. Depth 3 hides the
            # matmul->exp->psum-free turnaround that a 2-deep ring cannot.
            with (
                tc.tile_pool(name="ps_str", bufs=3, space="PSUM") as ps_str,
                tc.tile_pool(name="ps_acc", bufs=1, space="PSUM") as ps_acc,
                tc.tile_pool(name="expt_pool",
                             bufs=max(TRAIL, TRAIL0) + POP_HOLD
                             + EXTRA_EXPT) as expt_pool,
                tc.tile_pool(name="expp_pool", bufs=2) as expp_pool,
                tc.tile_pool(name="rc_pool", bufs=4) as rc_pool,
                tc.tile_pool(name="xv_pool", bufs=8) as xv_pool,
                tc.tile_pool(name="xq2_pool", bufs=7) as xq2_pool,
                tc.tile_pool(name="xk2_pool", bufs=12) as xk2_pool,
                tc.tile_pool(name="out_pool", bufs=4) as out_pool,
            ):
                # ctx accumulator: 2 banks; slice (qi, h) = [128q, 65].
                # 12 interleaved accumulation chains share the banks, so
                # matmul start=True (bank-granular zeroing) cannot be used:
                # the tile is DVE-memset per sq-block and every ctx matmul
                # accumulates with start=False.
                # [128, qi, 256]: each qi stride 1KB, 2 qi per psum bank.
                # Explicitly memset between blocks (on DVE, in-order after
                # the norm reads) -- matmul start=True bank-zeroing would
                # race with the deferred norm reads of the previous block.
                psc = ps_acc.tile([128, SQB // 128, 256], f32, name="psc")
                nc.vector.memset(psc[:], 0.0)

                # ---- background projections, chunk-granular ----
                # each (c, grp) group is split into a matmul op and a
                # deferred Act epilogue so the Act in-order queue never
                # parks on a bias-add whose matmuls are still running
                def make_bg_proj(xin, wsb, bsb, dst, sq, dma_eng, pool):
                    mm_ops = []
                    epi_ops = []
                    state = {}

                    def mk_dma(ech):
                        def op():
                            xt2 = pool.tile([128, QT], bf16, tag="xt2",
                                            name="xt2")
                            dma_eng.dma_start(
                                xt2[:],
                                xin[ech * 128:(ech + 1) * 128,
                                    sq * QT:(sq + 1) * QT],
                            )
                            state[ech] = xt2
                        return op

                    def mk_group(c, grp):
                        def mm_op():
                            pt = ps_str.tile([128, 512], f32, tag="u",
                                             name="pq")
                            rows = 128 if grp == 0 else 64
                            wc = slice(0, 128) if grp == 0 else slice(128, CL)
                            for ech in range(6):
                                nc.tensor.matmul(
                                    pt[0:rows, :], wsb[:, ech, wc],
                                    state[ech][:, c * 512:(c + 1) * 512],
                                    start=(ech == 0), stop=(ech == 5),
                                )
                            state[("pt", c, grp)] = pt

                        def epi_op():
                            pt = state.pop(("pt", c, grp))
                            qs = slice(sq * QT + c * 512,
                                       sq * QT + (c + 1) * 512)
                            if grp == 0:
                                nc.scalar.add(
                                    dst[:, 0, qs], pt[:, :], bsb[:, 0:1])
                            else:
                                nc.scalar.add(
                                    dst[0:64, 1, qs], pt[0:64, :],
                                    bsb[0:64, 1:2])
                        return mm_op, epi_op

                    for ech in range(6):
                        mm_ops.append(mk_dma(ech))
                    for c in range(QT // 512):
                        for grp in range(2):
                            mm, epi = mk_group(c, grp)
                            mm_ops.append(mm)
                            epi_ops.append(epi)
                    return mm_ops, epi_ops

                bg_work = []
                # k blocks 1..: block b first needed at pair-slot 12b
                for sq in range(1, NQB):
                    base = 12 * (sq - 1)
                    mm_ops, epi_ops = make_bg_proj(
                        xkT, wk_sb, bk_sb, kT8_sb, sq, nc.sync, xk2_pool)
                    tags = [base + i for i in range(6)] + \
                           [base + KG_OFF + i for i in range(4)]
                    for tg, op in zip(tags, mm_ops):
                        bg_work.append((tg, op))
                    for i, op in enumerate(epi_ops):
                        bg_work.append((base + KG_OFF + i + EPI_DELAY, op))
                # q blocks 1..: block n needed by pair-slot 96n
                for sq in range(1, NQB):
                    t0 = max(14, 2 * sq * SLOTS_PER_SQB + QBG_OFF)
                    mm_ops, epi_ops = make_bg_proj(
                        xqT, wq_sb, bq_sb, qT8_sb, sq, nc.gpsimd, xq2_pool)
                    for i, op in enumerate(mm_ops):
                        bg_work.append((t0 + i, op))
                    for i, op in enumerate(epi_ops):
                        bg_work.append((t0 + 6 + i + EPI_DELAY, op))
                bg_work.sort(key=lambda x: x[0])

                # ---- V projection (natural orientation) ----
                xvts = {}

                def load_xv_chunk(sqx, ech, eng=None):
                    xvt = xv_pool.tile([128, QT], bf16, tag="xvt",
                                       name="xvt")
                    (eng or nc.gpsimd).dma_start(
                        xvt[:],
                        xvT[ech * 128:(ech + 1) * 128,
                            sqx * QT:(sqx + 1) * QT],
                    )
                    xvts[(sqx, ech)] = xvt

                def emit_vproj_pair(stp):
                    # two s-tiles (2*stp, 2*stp+1) share one psum tile
                    pv = ps_str.tile([128, 2, 512], f32, tag="u", name="pv")
                    for half in range(2):
                        st = 2 * stp + half
                        sqx, stl = st // (QT // 128), st % (QT // 128)
                        if st == 0:
                            for ech in range(6):
                                load_xv_chunk(0, ech, nc.sync)
                        if stl < 6 and sqx + 1 < NQB:
                            load_xv_chunk(sqx + 1, stl)
                        for ech in range(6):
                            nc.tensor.matmul(
                                pv[:, half, 0:CL],
                                xvts[(sqx, ech)][:,
                                                 stl * 128:(stl + 1) * 128],
                                wv_sb[:, ech, :],
                                start=(ech == 0), stop=(ech == 5),
                            )
                    for half in range(2):
                        st = 2 * stp + half
                        sqx, stl = st // (QT // 128), st % (QT // 128)
                        nc.vector.tensor_add(
                            v_sb[:, st, :, 0:D],
                            pv[:, half, 0:CL].rearrange(
                                "p (h d) -> p h d", h=HL),
                            bvb_sb[:].rearrange("p (h d) -> p h d", h=HL),
                        )
                        if stl == (QT // 128) - 1:
                            for ech in range(6):
                                del xvts[(sqx, ech)]

                # ---- prefix scores + exp for one sq-block (3 heads) ----
                expp_cur = [None]

                def emit_prefix(sqb):
                    psm = ps_str.tile([128, 512], f32, tag="u", name="psp")
                    qs = slice(sqb * SQB, (sqb + 1) * SQB)
                    # head h lives at partitions 32h..32h+16 (same rows as
                    # its scores psum), so one [80, 512] activation handles
                    # all three heads; rows 16-31/48-63 are junk but finite
                    ep = expp_pool.tile([128, SQB], bf16, tag="ep",
                                        name="ep")
                    for h in range(HL):
                        pr, po = h // 2, 64 * (h % 2)
                        nc.tensor.matmul(
                            psm[32 * h:32 * h + PP, :],
                            kpT_sb[po:po + 64, pr, :],
                            qT8_sb[po:po + 64, pr, qs],
                            start=True, stop=True,
                        )
                    if sqb == 0:
                        # ring psum is uninitialized at t=0: only touch the
                        # rows the matmuls wrote
                        for h in range(HL):
                            nc.scalar.activation(
                                ep[32 * h:32 * h + PP, :],
                                psm[32 * h:32 * h + PP, :],
                                EXP, scale=0.125)
                    else:
                        nc.scalar.activation(
                            ep[0:80, :], psm[0:80, :], EXP, scale=0.125)
                    expp_cur[0] = ep

                # ---- scores (DoubleRow fp8) + exp for one (sqb, kp, h) ----
                # dynamic Schraudolph share: Act absorbs more exp where DVE
                # is congested (right after a block's norm) and in the
                # PE-bound first block; DVE takes more mid-block
                dve_acc = [0.0]

                def f_dve_at(t):
                    if t < SLOTS_PER_SQB + 6:
                        return F_SQB0
                    p = t % SLOTS_PER_SQB
                    if WLO <= p < WHI:
                        return F_LOW
                    return F_DVE

                def emit_scores_exp(t, sqb, kp, h):
                    pr, po = h // 2, 64 * (h % 2)
                    qs = slice(sqb * SQB, (sqb + 1) * SQB)
                    pss = ps_str.tile([128, 2, 512], f32, tag="u",
                                      name="pss")
                    qmov = qT8_sb[po:po + 64, pr:pr + 1, qs].to_broadcast(
                        (64, 2, SQB))
                    for i in range(2):
                        kt = 2 * kp + i
                        nc.tensor.matmul(
                            pss[:, i, :],
                            kT8_sb[po:po + 64, pr:pr + 1,
                                   kt * 128:(kt + 1) * 128].to_broadcast(
                                (64, 2, 128)),
                            qmov,
                            start=True, stop=True,
                            perf_mode=DR,
                        )
                    expt = expt_pool.tile([128, 2, 512], bf16, tag="expt",
                                          name="expt")
                    # near-alternating engine assignment so consecutive exps
                    # overlap across ScalarE / DVE
                    use_dve = False
                    dve_acc[0] += f_dve_at(t)
                    if dve_acc[0] >= 1.0:
                        dve_acc[0] -= 1.0
                        use_dve = True
                    if use_dve:
                        nc.vector.tensor_scalar(
                            expt[:].bitcast(i16), pss[:], SCH_A, SCH_B,
                            MULT, ADD)
                    else:
                        nc.scalar.activation(expt[:], pss[:], EXP,
                                             scale=0.0625)
                    return expt

                # ---- ctx (natural orientation) ----
                def emit_ctx(sqb, kp, h, expt, ep):
                    for qi in range(SQB // 128):
                        acc = psc[:, qi, 65 * h:65 * h + 65]
                        if kp == 0:
                            nc.tensor.matmul(
                                acc,
                                ep[32 * h:32 * h + PP,
                                   qi * 128:(qi + 1) * 128],
                                vp_sb[32 * h:32 * h + PP, h, :],
                                start=False, stop=False,
                                skip_group_check=True,
                            )
                        for i in range(2):
                            kt = 2 * kp + i
                            nc.tensor.matmul(
                                acc,
                                expt[:, i, qi * 128:(qi + 1) * 128],
                                v_sb[:, kt, h, :],
                                start=False, stop=False,
                                skip_group_check=True,
                            )

                # ---- norm + transpose + out-projection per sq-block ----
                outproj_work = []

                norm_work = []

                def emit_norm_qi(sqb, qi):
                    pscq = psc[:, qi, 0:195].rearrange(
                        "p (h c) -> p h c", c=65)
                    rc = rc_pool.tile([128, 4], f32, tag="rc", name="rc")
                    nc.vector.reciprocal(
                        rc[:, 0:HL], pscq[:, :, D])
                    for h in range(HL):
                        nc.vector.tensor_scalar_mul(
                            ctxn_sb[:, qi, h, :],
                            pscq[:, h, 0:D],
                            rc[:, h:h + 1])
                    qs = slice(sqb * SQB + qi * 128,
                               sqb * SQB + (qi + 1) * 128)
                    nc.sync.dma_start(
                        ctxT_sb[:, 0, qs], ctxn_sb[:, qi, 0:2, :],
                        transpose=True)
                    nc.sync.dma_start(
                        ctxT_sb[:, 1, qs], ctxn_sb[:, qi, 2:4, :],
                        transpose=True)
                    if qi == SQB // 128 - 1:
                        nc.vector.memset(psc[:], 0.0)
                        # delay the out-projection matmuls so their
                        # transpose dependencies clear before they enter
                        # PE's in-order queue
                        for et in range(6):
                            outproj_work.append(
                                (cur_t[0] + OP_DELAY + OP_SP * et, et, sqb))

                def emit_norm(sqb):
                    for qi in range(SQB // 128):
                        norm_work.append((cur_t[0] + qi, sqb, qi))

                def emit_outproj_tile(due, et, sqb):
                    es = slice(et * 128, (et + 1) * 128)
                    qs = slice(sqb * SQB, (sqb + 1) * SQB)
                    po3 = ps_str.tile([128, 512], f32, tag="u", name="po3")
                    for qi in range(SQB // 128):
                        ns = slice(qi * 128, (qi + 1) * 128)
                        qs2 = slice(sqb * SQB + qi * 128,
                                    sqb * SQB + (qi + 1) * 128)
                        nc.tensor.matmul(
                            po3[:, ns], wo_sb[:, 0, es], ctxT_sb[:, 0, qs2],
                            start=True, stop=False,
                        )
                        nc.tensor.matmul(
                            po3[:, ns], wo_sb[0:64, 1, es],
                            ctxT_sb[0:64, 1, qs2],
                            start=False, stop=True,
                        )
                    ot = out_pool.tile([128, 512], bf16, tag="ot", name="ot")
                    if OUT_DVE and sqb < NSQB - 1 and et % 2 == 0:
                        nc.vector.tensor_copy(ot[:], po3[:])
                    else:
                        nc.scalar.copy(ot[:], po3[:])
                    nc.gpsimd.dma_start(outT[es, qs], ot[:])

                # ---- the slot stream ----
                slots = [(sqb, kp, h)
                         for sqb in range(NSQB)
                         for kp in range(NKP)
                         for h in range(HL)]
                pending = []
                vst = 0
                cur_t = [0]
                hold_until = [INIT_HOLD]

                def pop_one():
                    (s2, e2, ep2) = pending.pop(0)
                    sqb2, kp2, h2 = s2
                    emit_ctx(sqb2, kp2, h2, e2, ep2)
                    if kp2 == NKP - 1 and h2 == HL - 1:
                        emit_norm(sqb2)
                        hold_until[0] = cur_t[0] + POP_HOLD
                        return True
                    return False

                for t, slot in enumerate(slots):
                    cur_t[0] = t
                    sqb, kp, h = slot
                    if kp == 0 and h == 0:
                        emit_prefix(sqb)
                    expt = emit_scores_exp(t, *slot)
                    pending.append((slot, expt, expp_cur[0]))
                    if vst < NST // 2 and t % VP_PACE == 0:
                        emit_vproj_pair(vst)
                        vst += 1
                    if t < 60:
                        trail_eff = TRAIL0
                    elif t < len(slots) - TAPER:
                        # per-block taper: drain the trail during each
                        # block's last slots (PE-light there) so the norm
                        # lands at the boundary, not 8 slots into the next
                        p = t % SLOTS_PER_SQB
                        if p >= TAPER_AT or p < TAPER_POST:
                            trail_eff = TRAIL_MIN
                        else:
                            trail_eff = TRAIL
                    else:
                        trail_eff = 1
                    if t >= hold_until[0]:
                        for _ in range(3):
                            if len(pending) > trail_eff:
                                # stop popping the moment a block ends: the
                                # next block's psc matmuls must not be
                                # emitted before the deferred norm + memset
                                if pop_one():
                                    break
                            else:
                                break
                    while norm_work and norm_work[0][0] <= t:
                        emit_norm_qi(*norm_work.pop(0)[1:])
                    while bg_work and bg_work[0][0] <= t:
                        bg_work.pop(0)[1]()
                    if outproj_work and outproj_work[0][0] <= t:
                        emit_outproj_tile(*outproj_work.pop(0))
                while pending:
                    pop_one()
                    while norm_work:
                        emit_norm_qi(*norm_work.pop(0)[1:])
                    if outproj_work:
                        emit_outproj_tile(*outproj_work.pop(0))
                while norm_work:
                    emit_norm_qi(*norm_work.pop(0)[1:])
                for _, op in bg_work:
                    op()
                while outproj_work:
                    emit_outproj_tile(*outproj_work.pop(0))

    nc.compile()
    return nc


def _get_nc():
    with _lock:
        if "nc" not in _compiled:
            _compiled["nc"] = _build()
        return _compiled["nc"]


def _prep_in_maps(query, key, value, prompt, Wq, bq, Wk, bk, Wv, bv, Wo, bo):
    f32 = np.float32
    qT = [np.ascontiguousarray(query[b].T).astype(BF16) for b in range(B)]
    kT = [np.ascontiguousarray(key[b].T).astype(BF16) for b in range(B)]
    vT = [np.ascontiguousarray(value[b].T).astype(BF16) for b in range(B)]
    in_maps = []
    for core in range(NCORES):
        b, g = core // NG, core % NG
        cs = slice(g * CL, (g + 1) * CL)
        kp = np.zeros((128, 2, PP), E4M3)
        vpa = np.zeros((128, HL, D + 1), BF16)
        for h in range(HL):
            gh = g * HL + h
            kp[64 * (h % 2):64 * (h % 2) + 64, h // 2, :] = (
                prompt[b, 0, :, gh, :].T.astype(E4M3))
            vpa[32 * h:32 * h + PP, h, D] = 1.0
            vpa[32 * h:32 * h + PP, h, 0:D] = (
                prompt[b, 1, :, gh, :].astype(BF16))
        in_maps.append({
            "xqT": qT[b], "xkT": kT[b], "xvT": vT[b],
            "wqT": np.ascontiguousarray(Wq[cs, :].T).astype(BF16),
            "wkT": np.ascontiguousarray(Wk[cs, :].T).astype(BF16),
            "wvT": np.ascontiguousarray(Wv[cs, :].T).astype(BF16),
            "woT": np.ascontiguousarray(Wo[:, cs].T).astype(BF16),
            "bq": np.ascontiguousarray(bq[cs]).astype(f32).reshape(CL, 1),
            "bk": np.ascontiguousarray(bk[cs]).astype(f32).reshape(CL, 1),
            "bv": np.ascontiguousarray(bv[cs]).astype(f32).reshape(1, CL),
            "kpT": kp, "vp": vpa,
        })
    return in_maps


def _combine(results, bo):
    out = np.empty((B, S, E), np.float32)
    for b in range(B):
        acc = results[b * NG]["outT"].astype(np.float32)
        for g in range(1, NG):
            acc = acc + results[b * NG + g]["outT"].astype(np.float32)
        out[b] = acc.T
    if bo is not None and np.any(bo):
        out += np.asarray(bo, np.float32)
    return out


def run(inputs, trace=False):
    """Returns (output, exec_time_ns or None)."""
    from concourse import bass_utils

    nc = _get_nc()
    in_maps = _prep_in_maps(**{k: np.asarray(v) for k, v in inputs.items()})
    bo = np.asarray(inputs["bo"])
    res = bass_utils.run_bass_kernel_spmd(
        nc, in_maps, core_ids=list(range(NCORES)), trace=trace,
    )
    return _combine(res.results, bo), res.exec_time_ns


def kernel(**inputs):
    out, _ = run(inputs)
    return out
